# revision 8
# baseline (speedup 1.0000x reference)
"""Trainium2 Bass kernel for a GAT block (GATConv + LN + FFN + LN).

v2: partition-aligned destination scheme.

Per-core plan (identity node order; core c owns nodes [OWN*c, OWN*(c+1))):
  Phase A: hp = x @ [W | W@Asrc | W@Adst] for all 50176 nodes ([128,144] psum
           per tile); rows [h(128) | a_src(8) | pad] stored to h_d (512B bf16
           rows, +1 row offset; row 0 / row NP+1 are pad rows with
           a_src = -200 so padded gather slots contribute exp(...)=~0).
           a_dst for own nodes kept in SBUF (ad_sb [128, NBLK, 8] bf16) and
           also written to ad_d [OWN,128] for the spill path.
  Phase B: edges partitioned by dst-block (128 own nodes per block).
           ALIGNED streams (AL/AH = src row below/above 32767): granule g of
           block b holds, at partition p, the g-th in-edge of node p. One
           512B-row dma_gather per slot brings [h | a_src]; a_dst is read
           directly from ad_sb (lane == dst). p = exp(leaky(a_src+a_dst));
           pexp (p broadcast over F) is written by the Act engine into the
           pad half of the gathered rows so msg = h*pexp is a fully-packed
           bf16 DVE op (2x mode). Scatter = psum accumulate with lhsT=I.
           SPILL streams (SL/SH): per-(block,stream) overflow edges beyond
           the per-lane cap, dst-grouped; S built via is_equal with a
           middle-broadcast AP (2x mode), a_dst gathered from ad_d;
           scatter matmul with lhsT = S[:, :, g].
  Phase C: deferred, batched: per-block bn_stats/apply with the two Sqrt ops
           batched over all blocks (2 act-table loads total).
"""
import numpy as np
import ml_dtypes

N = 50000
NCORES = 8
OWN = 6272             # nodes per core (49 blocks of 128)
NP = OWN * NCORES      # 50176
BLK = 128
NBLK = OWN // BLK      # 49
H, F, D = 8, 16, 128
LN_EPS = 1e-5

LO_SPLIT = 32767       # src node < LO_SPLIT -> lo stream (h_d row = src+1)
HD_ROWS = 50432        # h_d rows: 0 pad_lo, 1..NP nodes, NP+1 pad_hi
PAD_HI_IDX = NP + 1 - 32768   # 17409
AL_GPC = 24            # granules per aligned chunk (3072 slots)
SP_GPC = 8             # granules per spill chunk (1024 slots)
KAL, KSP = 2.2, 4.7    # per-slot cost weights for cap optimization
PAD_DL = 200.0

bf16 = ml_dtypes.bfloat16


def _wrap16(idx):
    L = idx.shape[0]
    w = idx.reshape(L // 16, 16).T.astype(np.int16)
    return np.tile(w, (8, 1))                      # [128, L/16]


def _bfr(x):
    return np.ascontiguousarray(np.asarray(x, dtype=np.float32).astype(bf16))


def _build_host_data(inputs):
    x = np.asarray(inputs["x"], np.float32)
    W = np.asarray(inputs["W_gat"], np.float32)
    att_src = np.asarray(inputs["att_src"], np.float32)
    att_dst = np.asarray(inputs["att_dst"], np.float32)
    ei = np.asarray(inputs["edge_index"])

    src = ei[0].astype(np.int64)
    dst = ei[1].astype(np.int64)
    loops = np.arange(N, dtype=np.int64)
    src = np.concatenate([src, loops])
    dst = np.concatenate([dst, loops])

    # ---- per-core edge tables (identity node order) ----
    deg = np.zeros((NCORES, NBLK, BLK, 2), dtype=np.int32)
    core_e = []
    for c in range(NCORES):
        m = (dst >= OWN * c) & (dst < min(OWN * (c + 1), N))
        e_src = src[m]
        d_l = dst[m] - OWN * c
        b = d_l >> 7
        lane = d_l & 127
        s = (e_src >= LO_SPLIT).astype(np.int64)
        np.add.at(deg, (c, b, lane, s), 1)
        core_e.append((e_src, d_l, b, lane, s))

    # ---- shared caps per (block, stream); spill granule profile ----
    caps = np.zeros((NBLK, 2), dtype=np.int64)
    spg = np.zeros((NBLK, 2), dtype=np.int64)
    for b in range(NBLK):
        for s in range(2):
            d = deg[:, b, :, s]                       # [cores, 128]
            dmax = int(d.max())
            caps_r = np.arange(dmax + 1)
            spill = np.maximum(d[:, :, None] - caps_r[None, None, :], 0
                               ).sum(axis=1).max(axis=0)     # [dmax+1]
            g_sp = -(-spill // 128)
            cost = 128 * caps_r * KAL + 128 * g_sp * KSP
            k = int(np.argmin(cost))
            caps[b, s] = k
            spg[b, s] = g_sp[k]

    albase = np.zeros((NBLK, 2), dtype=np.int64)
    spbase = np.zeros((NBLK, 2), dtype=np.int64)
    albase[1:] = np.cumsum(caps[:-1], axis=0)
    spbase[1:] = np.cumsum(spg[:-1], axis=0)
    G_AL = [int(caps[:, s].sum()) for s in range(2)]
    G_SP = [int(spg[:, s].sum()) for s in range(2)]
    # pad granule counts to chunk multiples
    G_ALp = [-(-g // AL_GPC) * AL_GPC if g else 0 for g in G_AL]
    G_SPp = [-(-g // SP_GPC) * SP_GPC if g else 0 for g in G_SP]
    L_AL = [g * 128 for g in G_ALp]
    L_SP = [g * 128 for g in G_SPp]

    # block-of-granule maps (shared); pad granules -> block 0
    blk_of_g = []
    for s in range(2):
        bg = np.repeat(np.arange(NBLK), caps[:, s])
        bg = np.concatenate([bg, np.zeros(G_ALp[s] - len(bg), dtype=np.int64)])
        blk_of_g.append(bg)
    # runs per aligned chunk: list of (g0_local, g1_local, block)
    runs = []
    for s in range(2):
        rs = []
        bg = blk_of_g[s]
        for k in range(G_ALp[s] // AL_GPC):
            seg = bg[k * AL_GPC:(k + 1) * AL_GPC]
            r = []
            i = 0
            while i < AL_GPC:
                j = i
                while j < AL_GPC and seg[j] == seg[i]:
                    j += 1
                r.append((i, j, int(seg[i])))
                i = j
            rs.append(r)
        runs.append(rs)

    # ---- per-core slot data ----
    per_core = []
    for c in range(NCORES):
        e_src, d_l, b, lane, s = core_e[c]
        key = (b * 128 + lane) * 2 + s
        order = np.argsort(key, kind="stable")
        ks = key[order]
        chg = np.r_[True, ks[1:] != ks[:-1]] if len(ks) else np.array([], bool)
        grp_id = np.cumsum(chg) - 1 if len(ks) else ks
        starts = np.flatnonzero(chg)
        occ = np.arange(len(ks)) - starts[grp_id] if len(ks) else ks
        so, bo, lo_, eo = s[order], b[order], lane[order], e_src[order]
        dlo = d_l[order]
        cap_e = caps[bo, so]
        al_m = occ < cap_e

        enc = np.where(so == 0, eo + 1, eo + 1 - 32768).astype(np.int64)

        gidx_al = [np.zeros(L_AL[0], dtype=np.int64),
                   np.full(L_AL[1], PAD_HI_IDX, dtype=np.int64)]
        for s_ in range(2):
            mm = al_m & (so == s_)
            g = albase[bo[mm], s_] + occ[mm]
            slot = g * 128 + lo_[mm]
            gidx_al[s_][slot] = enc[mm]

        gidx_sp = [np.zeros(L_SP[0], dtype=np.int64),
                   np.full(L_SP[1], PAD_HI_IDX, dtype=np.int64)]
        dl_sp = [np.full(L_SP[s_], PAD_DL, dtype=np.float32) for s_ in range(2)]
        aidx_sp = [np.zeros(L_SP[s_], dtype=np.int64) for s_ in range(2)]
        sp_m = ~al_m
        key2 = bo[sp_m] * 2 + so[sp_m]
        order2 = np.argsort(key2, kind="stable")
        k2 = key2[order2]
        chg2 = np.r_[True, k2[1:] != k2[:-1]] if len(k2) else np.array([], bool)
        gid2 = np.cumsum(chg2) - 1 if len(k2) else k2
        st2 = np.flatnonzero(chg2)
        rank2 = np.arange(len(k2)) - st2[gid2] if len(k2) else k2
        b2 = bo[sp_m][order2]
        s2 = so[sp_m][order2]
        e2 = enc[sp_m][order2]
        lane2 = lo_[sp_m][order2]
        dl2 = dlo[sp_m][order2]
        for s_ in range(2):
            mm = s2 == s_
            slot = spbase[b2[mm], s_] * 128 + rank2[mm]
            gidx_sp[s_][slot] = e2[mm]
            dl_sp[s_][slot] = lane2[mm].astype(np.float32)
            aidx_sp[s_][slot] = dl2[mm]

        ent = {}
        for s_, nm in ((0, "l"), (1, "h")):
            if L_AL[s_]:
                ent[f"gidx_a{nm}"] = _wrap16(gidx_al[s_])
            if L_SP[s_]:
                ent[f"gidx_s{nm}"] = _wrap16(gidx_sp[s_])
                ent[f"aidx_s{nm}"] = _wrap16(aidx_sp[s_])
                ent[f"dl_s{nm}"] = np.ascontiguousarray(
                    dl_sp[s_].astype(bf16).reshape(-1, 128).T)
        per_core.append(ent)

    # ---- weights / constants ----
    Asrc = np.zeros((D, H), np.float32)
    Adst = np.zeros((D, H), np.float32)
    for h in range(H):
        Asrc[h * F:(h + 1) * F, h] = att_src[h]
        Adst[h * F:(h + 1) * F, h] = att_dst[h]
    Wp = _bfr(np.concatenate([W, W @ Asrc, W @ Adst], axis=1))   # [128,144]
    I128 = _bfr(np.eye(128, dtype=np.float32))
    iota8 = _bfr(np.tile(np.arange(BLK, dtype=np.float32)[None, :, None],
                         (128, 1, SP_GPC)).reshape(128, BLK * SP_GPC))

    xp = np.zeros((NP, D), np.float32)
    xp[:N] = x
    xT = np.ascontiguousarray(xp.T.astype(bf16))                 # [128, NP]
    x_own = [np.ascontiguousarray(xp[OWN * c: OWN * (c + 1)]) for c in range(NCORES)]

    host = {
        "caps": caps, "spg": spg, "albase": albase, "spbase": spbase,
        "G_ALp": G_ALp, "G_SPp": G_SPp, "L_AL": L_AL, "L_SP": L_SP,
        "runs": runs, "per_core": per_core,
        "xT": xT, "x_own": x_own, "Wp": Wp, "I128": I128, "iota8": iota8,
        "W1": _bfr(np.asarray(inputs["w_ff1"], np.float32)),     # [128,256]
        "W2": _bfr(np.asarray(inputs["w_ff2"], np.float32)),     # [256,128]
        "b1col": np.ascontiguousarray(
            np.asarray(inputs["b_ff1"], np.float32).reshape(2, 128).T),  # [128,2]
    }
    host["bias_gat"] = np.asarray(inputs["bias_gat"], np.float32)
    host["b_ff2"] = np.asarray(inputs["b_ff2"], np.float32)
    for nm in ("gamma1", "beta1", "gamma2", "beta2"):
        host[nm] = np.asarray(inputs[nm], np.float32)
    host["triv_gb1"] = bool(np.all(host["gamma1"] == 1) and np.all(host["beta1"] == 0))
    host["triv_gb2"] = bool(np.all(host["gamma2"] == 1) and np.all(host["beta2"] == 0))
    host["triv_bgat"] = bool(np.all(host["bias_gat"] == 0))
    host["triv_bff2"] = bool(np.all(host["b_ff2"] == 0))
    return host


def _build_program(host):
    import concourse.bacc as bacc
    import concourse.mybir as mybir
    import concourse.tile as tile
    from concourse.bass import AP

    fp32 = mybir.dt.float32
    bft = mybir.dt.bfloat16
    i16 = mybir.dt.int16
    Alu = mybir.AluOpType
    Act = mybir.ActivationFunctionType

    caps, spg = host["caps"], host["spg"]
    albase, spbase = host["albase"], host["spbase"]
    L_AL, L_SP = host["L_AL"], host["L_SP"]
    runs = host["runs"]

    nc = bacc.Bacc("TRN2")

    # ---- DRAM tensors ----
    xT_d = nc.dram_tensor("xT", [128, NP], bft, kind="ExternalInput")
    xown_d = nc.dram_tensor("x_own", [OWN, D], fp32, kind="ExternalInput")
    Wp_d = nc.dram_tensor("Wp", [128, 144], bft, kind="ExternalInput")
    I128_d = nc.dram_tensor("I128", [128, 128], bft, kind="ExternalInput")
    iota8_d = nc.dram_tensor("iota8", [128, BLK * SP_GPC], bft, kind="ExternalInput")
    W1_d = nc.dram_tensor("W1", [128, 256], bft, kind="ExternalInput")
    W2_d = nc.dram_tensor("W2", [256, 128], bft, kind="ExternalInput")
    b1c_d = nc.dram_tensor("b1col", [128, 2], fp32, kind="ExternalInput")
    gl_d = {}
    if not host["triv_bgat"]:
        gl_d["bgat"] = nc.dram_tensor("bgat_r", [128, 128], fp32, kind="ExternalInput")
    if not host["triv_bff2"]:
        gl_d["bff2"] = nc.dram_tensor("bff2_r", [128, 128], fp32, kind="ExternalInput")
    if not host["triv_gb1"]:
        gl_d["g1"] = nc.dram_tensor("g1_r", [128, 128], fp32, kind="ExternalInput")
        gl_d["b1"] = nc.dram_tensor("b1_r", [128, 128], fp32, kind="ExternalInput")
    if not host["triv_gb2"]:
        gl_d["g2"] = nc.dram_tensor("g2_r", [128, 128], fp32, kind="ExternalInput")
        gl_d["b2"] = nc.dram_tensor("b2_r", [128, 128], fp32, kind="ExternalInput")

    sd = {}
    for s, nm in ((0, "l"), (1, "h")):
        if L_AL[s]:
            sd[f"gidx_a{nm}"] = nc.dram_tensor(
                f"gidx_a{nm}", [128, L_AL[s] // 16], i16, kind="ExternalInput")
        if L_SP[s]:
            sd[f"gidx_s{nm}"] = nc.dram_tensor(
                f"gidx_s{nm}", [128, L_SP[s] // 16], i16, kind="ExternalInput")
            sd[f"aidx_s{nm}"] = nc.dram_tensor(
                f"aidx_s{nm}", [128, L_SP[s] // 16], i16, kind="ExternalInput")
            sd[f"dl_s{nm}"] = nc.dram_tensor(
                f"dl_s{nm}", [128, L_SP[s] // 128], bft, kind="ExternalInput")

    h_d = nc.dram_tensor("h_scratch", [HD_ROWS, 256], bft, kind="Internal")
    ad_d = nc.dram_tensor("adst_scratch", [OWN, 128], bft, kind="Internal")
    z_d = nc.dram_tensor("z", [OWN, D], fp32, kind="ExternalOutput")

    h_lo = h_d[0:32768, :]
    h_hi = h_d[32768:HD_ROWS, :]

    NT = NP // 128                    # 392 node tiles
    with tile.TileContext(nc) as tc:
        # ================= consts =================
        cpool = tc.alloc_tile_pool(name="consts", bufs=1)
        Wp_s = cpool.tile([128, 144], bft)
        nc.sync.dma_start(out=Wp_s[:], in_=Wp_d[:])
        I128_s = cpool.tile([128, 128], bft)
        nc.sync.dma_start(out=I128_s[:], in_=I128_d[:])
        iota8_s = cpool.tile([128, BLK, SP_GPC], bft)
        nc.sync.dma_start(out=iota8_s[:], in_=iota8_d[:].rearrange(
            "p (n g) -> p n g", g=SP_GPC))
        W1_s = cpool.tile([128, 256], bft)
        nc.sync.dma_start(out=W1_s[:], in_=W1_d[:])
        W2_s = cpool.tile([256 // 2, 2, 128], bft)
        nc.sync.dma_start(out=W2_s[:],
                          in_=W2_d[:].rearrange("(k h) f -> h k f", k=2))
        b1c_s = cpool.tile([128, 2], fp32)
        nc.sync.dma_start(out=b1c_s[:], in_=b1c_d[:])
        gl_s = {}
        for k, dref in gl_d.items():
            gl_s[k] = cpool.tile([128, 128], fp32, tag=f"gl_{k}")
            nc.sync.dma_start(out=gl_s[k][:], in_=dref[:])
        eps_s = cpool.tile([128, 1], fp32)
        nc.vector.memset(eps_s[:], LN_EPS)
        ad_sb = cpool.tile([128, NBLK, 8], bft)

        # pad rows for h_d (row 0 and row NP+1): zeros, a_src cols = -200
        padr = cpool.tile([128, 256], bft)
        nc.vector.memset(padr[:], 0.0)
        nc.vector.memset(padr[:, 128:136], -200.0)
        nc.sync.dma_start(out=h_d[0:1, :], in_=padr[0:1, :])
        nc.sync.dma_start(out=h_d[NP + 1:NP + 2, :], in_=padr[0:1, :])

        # persistent phase-C tiles
        xo_all = cpool.tile([128, NBLK, 128], fp32)   # xo -> t1 -> t2 (reused)
        u_all = cpool.tile([128, NBLK, 128], fp32)
        mv1 = cpool.tile([128, NBLK, 2], fp32)
        mv2 = cpool.tile([128, NBLK, 2], fp32)
        sc1 = cpool.tile([128, NBLK], fp32)
        sc2 = cpool.tile([128, NBLK], fp32)
        nc.sync.dma_start(out=xo_all[:],
                          in_=xown_d[:].rearrange("(j n) d -> n j d", j=NBLK))

        # ================= phase A =================
        with tc.tile_pool(name="pA", bufs=4) as pA, \
             tc.tile_pool(name="psA", bufs=4, space="PSUM") as psA:
            GT = 3
            XB = 12
            xt = None
            for tg in range((NT + GT - 1) // GT):
                t0 = tg * GT
                ntl = min(GT, NT - t0)
                if t0 % XB == 0:
                    nxb = min(XB, NT - t0)
                    xt = pA.tile([128, XB * 128], bft, tag="xt")
                    nc.sync.dma_start(out=xt[:, :nxb * 128],
                                      in_=xT_d[:, t0 * 128:(t0 + nxb) * 128])
                ps = psA.tile([128, GT, 144], fp32, tag="psA")
                for j in range(ntl):
                    jo = (t0 % XB) + j
                    nc.tensor.matmul(ps[:, j, :],
                                     lhsT=xt[:, jo * 128:(jo + 1) * 128],
                                     rhs=Wp_s[:], start=True, stop=True)
                stage = pA.tile([128, GT, 256], bft, tag="stage")
                if tg % 2 == 0:
                    nc.scalar.activation(out=stage[:, :ntl, 0:144],
                                         in_=ps[:, :ntl, :], func=Act.Copy)
                else:
                    nc.vector.tensor_copy(out=stage[:, :ntl, 0:144],
                                          in_=ps[:, :ntl, :])
                nc.sync.dma_start(
                    out=h_d[t0 * 128 + 1:(t0 + ntl) * 128 + 1, :].rearrange(
                        "(j n) d -> n j d", j=ntl),
                    in_=stage[:, :ntl, :])
                if t0 < NBLK:
                    nob = min(ntl, NBLK - t0)
                    nc.vector.tensor_copy(out=ad_sb[:, t0:t0 + nob, :],
                                          in_=stage[:, :nob, 136:144])
                    for j in range(nob):
                        t = t0 + j
                        nc.sync.dma_start(out=ad_d[t * 128:(t + 1) * 128, 0:8],
                                          in_=stage[:, j, 136:144])

        tc.strict_bb_all_engine_barrier()

        # ================= phase B =================
        pB = tc.alloc_tile_pool(name="pB", bufs=2)
        pBs = tc.alloc_tile_pool(name="pBsmall", bufs=4)
        psB = tc.alloc_tile_pool(name="psB", bufs=2, space="PSUM")
        pC = tc.alloc_tile_pool(name="pC", bufs=2)
        psC = tc.alloc_tile_pool(name="psC", bufs=2, space="PSUM")

        SCFG = {
            "AL": dict(gpc=AL_GPC, al=True, src=h_lo, gx="gidx_al", s=0),
            "AH": dict(gpc=AL_GPC, al=True, src=h_hi, gx="gidx_ah", s=1),
            "SL": dict(gpc=SP_GPC, al=False, src=h_lo, gx="gidx_sl",
                       ax="aidx_sl", dl="dl_sl", s=0),
            "SH": dict(gpc=SP_GPC, al=False, src=h_hi, gx="gidx_sh",
                       ax="aidx_sh", dl="dl_sh", s=1),
        }
        chunk_tiles = {st: {} for st in SCFG}

        def emit_chunk(st, k):
            ct = chunk_tiles[st]
            if k in ct:
                return ct[k]
            cfg = SCFG[st]
            gpc = cfg["gpc"]
            ch = gpc * 128
            gix = pBs.tile([128, ch // 16], i16, tag=f"gix{st}")
            nc.sync.dma_start(
                out=gix[:],
                in_=sd[cfg["gx"]][:, k * (ch // 16):(k + 1) * (ch // 16)])
            h_ch = pB.tile([128, gpc, 256], bft, tag=f"h{st}")
            nc.gpsimd.dma_gather(h_ch[:], cfg["src"], gix[:], ch, ch, 256,
                                 single_packet=False)
            eL = pBs.tile([128, gpc, 8], bft, tag=f"eL{st}")
            res = {}
            if cfg["al"]:
                for (g0, g1, b) in runs[cfg["s"]][k]:
                    sl = ad_sb[:, b, :]
                    ad_b = AP(sl.tensor, sl.offset,
                              [sl.ap[0], [0, g1 - g0], sl.ap[1]])
                    nc.vector.tensor_tensor(out=eL[:, g0:g1, :],
                                            in0=h_ch[:, g0:g1, 128:136],
                                            in1=ad_b, op=Alu.add)
            else:
                aix = pBs.tile([128, ch // 16], i16, tag=f"aix{st}")
                nc.sync.dma_start(
                    out=aix[:],
                    in_=sd[cfg["ax"]][:, k * (ch // 16):(k + 1) * (ch // 16)])
                adE = pB.tile([128, gpc, 128], bft, tag=f"adE{st}")
                nc.gpsimd.dma_gather(adE[:], ad_d[:], aix[:], ch, ch, 128,
                                     single_packet=False)
                nc.vector.tensor_tensor(out=eL[:], in0=h_ch[:, :, 128:136],
                                        in1=adE[:, :, 0:8], op=Alu.add)
                dlt = pBs.tile([128, gpc], bft, tag=f"dl{st}")
                nc.sync.dma_start(out=dlt[:],
                                  in_=sd[cfg["dl"]][:, k * gpc:(k + 1) * gpc])
                S2 = pB.tile([128, BLK, gpc], bft, tag=f"S2{st}")
                dsl = dlt[:]
                dl_b = AP(dsl.tensor, dsl.offset,
                          [dsl.ap[0], [0, BLK], dsl.ap[1]])
                nc.vector.tensor_tensor(out=S2[:], in0=dl_b,
                                        in1=iota8_s[:, :, 0:gpc],
                                        op=Alu.is_equal)
                res["S2"] = S2
            eL2 = pBs.tile([128, gpc, 8], bft, tag=f"eL2{st}")
            nc.vector.scalar_tensor_tensor(out=eL2[:], in0=eL[:], scalar=0.2,
                                           in1=eL[:], op0=Alu.mult, op1=Alu.max)
            msgp = pB.tile([128, gpc, 136], bft, tag=f"msgp{st}")
            nc.scalar.activation(out=msgp[:, :, 128:136], in_=eL2[:],
                                 func=Act.Exp)
            if bool(int(_os.environ.get("GAT_NO_PEXP", "0"))):
                nc.vector.tensor_tensor(
                    out=msgp[:, :, 0:128].rearrange("p g (h f) -> p g h f", f=F),
                    in0=h_ch[:, :, 0:128].rearrange("p g (h f) -> p g h f", f=F),
                    in1=msgp[:, :, 128:136].to_broadcast([128, gpc, 8, F]),
                    op=Alu.mult)
            else:
                nc.scalar.activation(
                    out=h_ch[:, :, 128:256].rearrange("p g (h f) -> p g h f", f=F),
                    in_=eL2[:].to_broadcast([128, gpc, 8, F]), func=Act.Exp)
                nc.vector.tensor_tensor(out=msgp[:, :, 0:128],
                                        in0=h_ch[:, :, 0:128],
                                        in1=h_ch[:, :, 128:256], op=Alu.mult)
            res["msgp"] = msgp
            ct[k] = res
            return res

        import os as _os
        no_spill = bool(int(_os.environ.get("GAT_NO_SPILL", "0")))
        no_al = bool(int(_os.environ.get("GAT_NO_AL", "0")))
        no_c = bool(int(_os.environ.get("GAT_NO_C", "0")))
        for b in range(NBLK):
            glist = []
            if not no_al:
                for s, st in ((0, "AL"), (1, "AH")):
                    for o in range(int(caps[b, s])):
                        glist.append((st, int(albase[b, s]) + o))
            if not no_spill:
                for s, st in ((0, "SL"), (1, "SH")):
                    for o in range(int(spg[b, s])):
                        glist.append((st, int(spbase[b, s]) + o))
            tot = len(glist)
            ps_blk = psB.tile([128, 136], fp32, tag="blk")
            for i, (st, gg) in enumerate(glist):
                cfg = SCFG[st]
                res = emit_chunk(st, gg // cfg["gpc"])
                gl = gg % cfg["gpc"]
                lhsT = I128_s[:] if cfg["al"] else res["S2"][:, :, gl]
                nc.tensor.matmul(ps_blk[:], lhsT=lhsT,
                                 rhs=res["msgp"][:, gl, :],
                                 start=(i == 0), stop=(i == tot - 1))
            # normalize: gt = agg/denom; t1 = x + gt (into xo_all)
            rec = pBs.tile([128, 8], fp32, tag="rec")
            nc.vector.reciprocal(out=rec[:], in_=ps_blk[:, 128:136])
            gt = pBs.tile([128, 128], fp32, tag="gt")
            nc.vector.tensor_tensor(
                out=gt[:].rearrange("p (h f) -> p h f", f=F),
                in0=ps_blk[:, 0:128].rearrange("p (h f) -> p h f", f=F),
                in1=rec[:].to_broadcast([128, 8, F]), op=Alu.mult)
            if not host["triv_bgat"]:
                nc.vector.tensor_tensor(out=gt[:], in0=gt[:],
                                        in1=gl_s["bgat"][:], op=Alu.add)
            nc.vector.tensor_tensor(out=xo_all[:, b, :], in0=xo_all[:, b, :],
                                    in1=gt[:], op=Alu.add)

        # ================= phase C (batched sweep) =================
        t1_all = xo_all
        if no_c:
            for b in range(NBLK):
                nc.sync.dma_start(out=z_d[b * 128:(b + 1) * 128, :],
                                  in_=t1_all[:, b, :])
        for b in range(NBLK if not no_c else 0):
            bst = pBs.tile([128, 6], fp32, tag="bst")
            nc.vector.bn_stats(out=bst[:], in_=t1_all[:, b, :])
            nc.vector.bn_aggr(out=mv1[:, b, :], in_=bst[:])
        if no_c:
            NBLK_C = 0
        else:
            NBLK_C = NBLK
            nc.scalar.activation(out=sc1[:], in_=mv1[:, :, 1], func=Act.Sqrt,
                                 bias=eps_s[:])
            nc.vector.reciprocal(out=sc1[:], in_=sc1[:])

        for b in range(NBLK_C):
            u = u_all[:, b, :]
            nc.vector.tensor_scalar(out=u, in0=t1_all[:, b, :],
                                    scalar1=mv1[:, b, 0:1], op0=Alu.subtract,
                                    scalar2=sc1[:, b:b + 1], op1=Alu.mult)
            if not host["triv_gb1"]:
                nc.vector.tensor_tensor(out=u, in0=u, in1=gl_s["g1"][:],
                                        op=Alu.mult)
                nc.vector.tensor_tensor(out=u, in0=u, in1=gl_s["b1"][:],
                                        op=Alu.add)
            u_bf = pC.tile([128, 128], bft, tag="ubf")
            nc.scalar.activation(out=u_bf[:], in_=u, func=Act.Copy)
            uT_ps = psC.tile([128, 128], bft, tag="uT")
            nc.tensor.transpose(uT_ps[:], in_=u_bf[:], identity=I128_s[:])
            uT = pC.tile([128, 128], bft, tag="uTs")
            nc.scalar.activation(out=uT[:], in_=uT_ps[:], func=Act.Copy)
            f1ps = psC.tile([128, 2, 128], fp32, tag="f1")
            for j in range(2):
                nc.tensor.matmul(f1ps[:, j, :], lhsT=W1_s[:, j * 128:(j + 1) * 128],
                                 rhs=uT[:], start=True, stop=True)
            r1 = pC.tile([128, 2, 128], bft, tag="r1")
            for j in range(2):
                nc.scalar.activation(out=r1[:, j, :], in_=f1ps[:, j, :],
                                     func=Act.Relu, bias=b1c_s[:, j:j + 1])
            zps = psC.tile([128, 128], fp32, tag="zp")
            for j in range(2):
                nc.tensor.matmul(zps[:], lhsT=r1[:, j, :], rhs=W2_s[:, j, :],
                                 start=(j == 0), stop=(j == 1))
            t2 = t1_all[:, b, :]
            nc.vector.tensor_tensor(out=t2, in0=u, in1=zps[:], op=Alu.add)
            if not host["triv_bff2"]:
                nc.vector.tensor_tensor(out=t2, in0=t2, in1=gl_s["bff2"][:],
                                        op=Alu.add)
            bst = pBs.tile([128, 6], fp32, tag="bst")
            nc.vector.bn_stats(out=bst[:], in_=t2)
            nc.vector.bn_aggr(out=mv2[:, b, :], in_=bst[:])
        if not no_c:
            nc.scalar.activation(out=sc2[:], in_=mv2[:, :, 1], func=Act.Sqrt,
                                 bias=eps_s[:])
            nc.vector.reciprocal(out=sc2[:], in_=sc2[:])

        for b in range(NBLK_C):
            zt = pC.tile([128, 128], fp32, tag="zt")
            nc.vector.tensor_scalar(out=zt[:], in0=t1_all[:, b, :],
                                    scalar1=mv2[:, b, 0:1], op0=Alu.subtract,
                                    scalar2=sc2[:, b:b + 1], op1=Alu.mult)
            if not host["triv_gb2"]:
                nc.vector.tensor_tensor(out=zt[:], in0=zt[:], in1=gl_s["g2"][:],
                                        op=Alu.mult)
                nc.vector.tensor_tensor(out=zt[:], in0=zt[:], in1=gl_s["b2"][:],
                                        op=Alu.add)
            nc.sync.dma_start(out=z_d[b * 128:(b + 1) * 128, :], in_=zt[:])

        for p in (psC, pC, psB, pBs, pB):
            p.release()
        cpool.release()

    nc.compile()
    return nc


def kernel(**inputs):
    import os
    from concourse.bass_utils import run_bass_kernel_spmd

    host = _build_host_data(inputs)
    nc = _build_program(host)

    in_maps = []
    for c in range(NCORES):
        m = {
            "xT": host["xT"],
            "x_own": host["x_own"][c],
            "Wp": host["Wp"], "I128": host["I128"], "iota8": host["iota8"],
            "W1": host["W1"], "W2": host["W2"], "b1col": host["b1col"],
        }
        if not host["triv_bgat"]:
            m["bgat_r"] = np.tile(host["bias_gat"].reshape(1, -1), (128, 1))
        if not host["triv_bff2"]:
            m["bff2_r"] = np.tile(host["b_ff2"].reshape(1, -1), (128, 1))
        if not host["triv_gb1"]:
            m["g1_r"] = np.tile(host["gamma1"].reshape(1, -1), (128, 1))
            m["b1_r"] = np.tile(host["beta1"].reshape(1, -1), (128, 1))
        if not host["triv_gb2"]:
            m["g2_r"] = np.tile(host["gamma2"].reshape(1, -1), (128, 1))
            m["b2_r"] = np.tile(host["beta2"].reshape(1, -1), (128, 1))
        for key in ("gidx_al", "gidx_ah", "gidx_sl", "gidx_sh",
                    "aidx_sl", "aidx_sh", "dl_sl", "dl_sh"):
            if key in host["per_core"][c]:
                m[key] = host["per_core"][c][key]
        in_maps.append(m)

    trace = bool(int(os.environ.get("GAT_TRACE", "0")))
    res = run_bass_kernel_spmd(nc, in_maps, core_ids=list(range(NCORES)),
                               trace=trace)
    if trace and res.exec_time_ns:
        print(f"HW exec time: {res.exec_time_ns} ns")
    if bool(int(os.environ.get("GAT_TIME", "0"))):
        try:
            from concourse.timeline_sim import TimelineSim
            ts = TimelineSim(nc)
            dur = ts.simulate()
            print(f"HW exec time: {dur:.0f} ns (cost-model timeline estimate)")
        except Exception as e:
            print("timeline sim failed:", e)

    out = np.zeros((N, D), np.float32)
    for c in range(NCORES):
        lo_n = OWN * c
        hi_n = min(OWN * (c + 1), N)
        out[lo_n:hi_n] = res.results[c]["z"][: hi_n - lo_n]
    return out


# revision 13
# speedup vs baseline: 1.0387x; 1.0387x over previous
"""Trainium2 Bass kernel for a GAT block (GATConv + LN + FFN + LN).

v2: partition-aligned destination scheme.

Per-core plan (identity node order; core c owns nodes [OWN*c, OWN*(c+1))):
  Phase A: hp = x @ [W | W@Asrc | W@Adst] for all 50176 nodes ([128,144] psum
           per tile); rows [h(128) | a_src(8) | pad] stored to h_d (512B bf16
           rows, +1 row offset; row 0 / row NP+1 are pad rows with
           a_src = -200 so padded gather slots contribute exp(...)=~0).
           a_dst for own nodes kept in SBUF (ad_sb [128, NBLK, 8] bf16) and
           also written to ad_d [OWN,128] for the spill path.
  Phase B: edges partitioned by dst-block (128 own nodes per block).
           ALIGNED streams (AL/AH = src row below/above 32767): granule g of
           block b holds, at partition p, the g-th in-edge of node p. One
           512B-row dma_gather per slot brings [h | a_src]; a_dst is read
           directly from ad_sb (lane == dst). p = exp(leaky(a_src+a_dst));
           pexp (p broadcast over F) is written by the Act engine into the
           pad half of the gathered rows so msg = h*pexp is a fully-packed
           bf16 DVE op (2x mode). Scatter = psum accumulate with lhsT=I.
           SPILL streams (SL/SH): per-(block,stream) overflow edges beyond
           the per-lane cap, dst-grouped; S built via is_equal with a
           middle-broadcast AP (2x mode), a_dst gathered from ad_d;
           scatter matmul with lhsT = S[:, :, g].
  Phase C: deferred, batched: per-block bn_stats/apply with the two Sqrt ops
           batched over all blocks (2 act-table loads total).
"""
import numpy as np
import ml_dtypes

N = 50000
NCORES = 8
OWN = 6272             # nodes per core (49 blocks of 128)
NP = OWN * NCORES      # 50176
BLK = 128
NBLK = OWN // BLK      # 49
H, F, D = 8, 16, 128
LN_EPS = 1e-5

LO_SPLIT = 32767       # src node < LO_SPLIT -> lo stream (h_d row = src+1)
HD_ROWS = 50432        # h_d rows: 0 pad_lo, 1..NP nodes, NP+1 pad_hi
PAD_HI_IDX = NP + 1 - 32768   # 17409
AL_GPC = 24            # granules per aligned chunk (3072 slots)
SP_GPC = 8             # granules per spill chunk (1024 slots)
KAL, KSP = 2.2, 4.7    # per-slot cost weights for cap optimization
PAD_DL = 200.0

bf16 = ml_dtypes.bfloat16


def _wrap16(idx):
    L = idx.shape[0]
    w = idx.reshape(L // 16, 16).T.astype(np.int16)
    return np.tile(w, (8, 1))                      # [128, L/16]


def _bfr(x):
    return np.ascontiguousarray(np.asarray(x, dtype=np.float32).astype(bf16))


def _build_host_data(inputs):
    x = np.asarray(inputs["x"], np.float32)
    W = np.asarray(inputs["W_gat"], np.float32)
    att_src = np.asarray(inputs["att_src"], np.float32)
    att_dst = np.asarray(inputs["att_dst"], np.float32)
    ei = np.asarray(inputs["edge_index"])

    src = ei[0].astype(np.int64)
    dst = ei[1].astype(np.int64)
    loops = np.arange(N, dtype=np.int64)
    src = np.concatenate([src, loops])
    dst = np.concatenate([dst, loops])

    # ---- per-core edge tables (identity node order) ----
    deg = np.zeros((NCORES, NBLK, BLK, 2), dtype=np.int32)
    core_e = []
    for c in range(NCORES):
        m = (dst >= OWN * c) & (dst < min(OWN * (c + 1), N))
        e_src = src[m]
        d_l = dst[m] - OWN * c
        b = d_l >> 7
        lane = d_l & 127
        s = (e_src >= LO_SPLIT).astype(np.int64)
        np.add.at(deg, (c, b, lane, s), 1)
        core_e.append((e_src, d_l, b, lane, s))

    # ---- shared caps per (block, stream); spill granule profile ----
    caps = np.zeros((NBLK, 2), dtype=np.int64)
    spg = np.zeros((NBLK, 2), dtype=np.int64)
    for b in range(NBLK):
        for s in range(2):
            d = deg[:, b, :, s]                       # [cores, 128]
            dmax = int(d.max())
            caps_r = np.arange(dmax + 1)
            spill = np.maximum(d[:, :, None] - caps_r[None, None, :], 0
                               ).sum(axis=1).max(axis=0)     # [dmax+1]
            g_sp = -(-spill // 128)
            cost = 128 * caps_r * KAL + 128 * g_sp * KSP
            k = int(np.argmin(cost))
            caps[b, s] = k
            spg[b, s] = g_sp[k]

    albase = np.zeros((NBLK, 2), dtype=np.int64)
    spbase = np.zeros((NBLK, 2), dtype=np.int64)
    albase[1:] = np.cumsum(caps[:-1], axis=0)
    spbase[1:] = np.cumsum(spg[:-1], axis=0)
    G_AL = [int(caps[:, s].sum()) for s in range(2)]
    G_SP = [int(spg[:, s].sum()) for s in range(2)]
    # pad granule counts to chunk multiples
    G_ALp = [-(-g // AL_GPC) * AL_GPC if g else 0 for g in G_AL]
    G_SPp = [-(-g // SP_GPC) * SP_GPC if g else 0 for g in G_SP]
    L_AL = [g * 128 for g in G_ALp]
    L_SP = [g * 128 for g in G_SPp]

    # block-of-granule maps (shared); pad granules -> block 0
    blk_of_g = []
    for s in range(2):
        bg = np.repeat(np.arange(NBLK), caps[:, s])
        bg = np.concatenate([bg, np.zeros(G_ALp[s] - len(bg), dtype=np.int64)])
        blk_of_g.append(bg)
    # runs per aligned chunk: list of (g0_local, g1_local, block)
    runs = []
    for s in range(2):
        rs = []
        bg = blk_of_g[s]
        for k in range(G_ALp[s] // AL_GPC):
            seg = bg[k * AL_GPC:(k + 1) * AL_GPC]
            r = []
            i = 0
            while i < AL_GPC:
                j = i
                while j < AL_GPC and seg[j] == seg[i]:
                    j += 1
                r.append((i, j, int(seg[i])))
                i = j
            rs.append(r)
        runs.append(rs)

    # ---- per-core slot data ----
    per_core = []
    for c in range(NCORES):
        e_src, d_l, b, lane, s = core_e[c]
        key = (b * 128 + lane) * 2 + s
        order = np.argsort(key, kind="stable")
        ks = key[order]
        chg = np.r_[True, ks[1:] != ks[:-1]] if len(ks) else np.array([], bool)
        grp_id = np.cumsum(chg) - 1 if len(ks) else ks
        starts = np.flatnonzero(chg)
        occ = np.arange(len(ks)) - starts[grp_id] if len(ks) else ks
        so, bo, lo_, eo = s[order], b[order], lane[order], e_src[order]
        dlo = d_l[order]
        cap_e = caps[bo, so]
        al_m = occ < cap_e

        enc = np.where(so == 0, eo + 1, eo + 1 - 32768).astype(np.int64)

        gidx_al = [np.zeros(L_AL[0], dtype=np.int64),
                   np.full(L_AL[1], PAD_HI_IDX, dtype=np.int64)]
        for s_ in range(2):
            mm = al_m & (so == s_)
            g = albase[bo[mm], s_] + occ[mm]
            slot = g * 128 + lo_[mm]
            gidx_al[s_][slot] = enc[mm]

        gidx_sp = [np.zeros(L_SP[0], dtype=np.int64),
                   np.full(L_SP[1], PAD_HI_IDX, dtype=np.int64)]
        dl_sp = [np.full(L_SP[s_], PAD_DL, dtype=np.float32) for s_ in range(2)]
        aidx_sp = [np.zeros(L_SP[s_], dtype=np.int64) for s_ in range(2)]
        sp_m = ~al_m
        key2 = bo[sp_m] * 2 + so[sp_m]
        order2 = np.argsort(key2, kind="stable")
        k2 = key2[order2]
        chg2 = np.r_[True, k2[1:] != k2[:-1]] if len(k2) else np.array([], bool)
        gid2 = np.cumsum(chg2) - 1 if len(k2) else k2
        st2 = np.flatnonzero(chg2)
        rank2 = np.arange(len(k2)) - st2[gid2] if len(k2) else k2
        b2 = bo[sp_m][order2]
        s2 = so[sp_m][order2]
        e2 = enc[sp_m][order2]
        lane2 = lo_[sp_m][order2]
        dl2 = dlo[sp_m][order2]
        for s_ in range(2):
            mm = s2 == s_
            slot = spbase[b2[mm], s_] * 128 + rank2[mm]
            gidx_sp[s_][slot] = e2[mm]
            dl_sp[s_][slot] = lane2[mm].astype(np.float32)
            aidx_sp[s_][slot] = dl2[mm]

        ent = {}
        for s_, nm in ((0, "l"), (1, "h")):
            if L_AL[s_]:
                ent[f"gidx_a{nm}"] = _wrap16(gidx_al[s_])
            if L_SP[s_]:
                ent[f"gidx_s{nm}"] = _wrap16(gidx_sp[s_])
                ent[f"aidx_s{nm}"] = _wrap16(aidx_sp[s_])
                ent[f"dl_s{nm}"] = np.ascontiguousarray(
                    dl_sp[s_].astype(bf16).reshape(-1, 128).T)
        per_core.append(ent)

    # ---- weights / constants ----
    Asrc = np.zeros((D, H), np.float32)
    Adst = np.zeros((D, H), np.float32)
    for h in range(H):
        Asrc[h * F:(h + 1) * F, h] = att_src[h]
        Adst[h * F:(h + 1) * F, h] = att_dst[h]
    Wp = _bfr(np.concatenate([W, W @ Asrc], axis=1))             # [128,136]
    Wad = _bfr(W @ Adst)                                         # [128,8]
    I128 = _bfr(np.eye(128, dtype=np.float32))
    iota8 = _bfr(np.tile(np.arange(BLK, dtype=np.float32)[None, :, None],
                         (128, 1, SP_GPC)).reshape(128, BLK * SP_GPC))

    xp = np.zeros((NP, D), np.float32)
    xp[:N] = x
    xT = np.ascontiguousarray(xp.T.astype(bf16))                 # [128, NP]
    x_own = [np.ascontiguousarray(xp[OWN * c: OWN * (c + 1)]) for c in range(NCORES)]
    x_ownT = [np.ascontiguousarray(xp[OWN * c: OWN * (c + 1)].T.astype(bf16))
              for c in range(NCORES)]

    host = {
        "caps": caps, "spg": spg, "albase": albase, "spbase": spbase,
        "G_ALp": G_ALp, "G_SPp": G_SPp, "L_AL": L_AL, "L_SP": L_SP,
        "runs": runs, "per_core": per_core,
        "xT": xT, "x_own": x_own, "x_ownT": x_ownT, "Wp": Wp, "Wad": Wad,
        "I128": I128, "iota8": iota8,
        "W1": _bfr(np.asarray(inputs["w_ff1"], np.float32)),     # [128,256]
        "W2": _bfr(np.asarray(inputs["w_ff2"], np.float32)),     # [256,128]
        "b1col": np.ascontiguousarray(
            np.asarray(inputs["b_ff1"], np.float32).reshape(2, 128).T),  # [128,2]
    }
    host["bias_gat"] = np.asarray(inputs["bias_gat"], np.float32)
    host["b_ff2"] = np.asarray(inputs["b_ff2"], np.float32)
    for nm in ("gamma1", "beta1", "gamma2", "beta2"):
        host[nm] = np.asarray(inputs[nm], np.float32)
    host["triv_gb1"] = bool(np.all(host["gamma1"] == 1) and np.all(host["beta1"] == 0))
    host["triv_gb2"] = bool(np.all(host["gamma2"] == 1) and np.all(host["beta2"] == 0))
    host["triv_bgat"] = bool(np.all(host["bias_gat"] == 0))
    host["triv_bff2"] = bool(np.all(host["b_ff2"] == 0))
    return host


def _build_program(host):
    import os as _os
    import concourse.bacc as bacc
    import concourse.mybir as mybir
    import concourse.tile as tile
    from concourse.bass import AP

    fp32 = mybir.dt.float32
    bft = mybir.dt.bfloat16
    i16 = mybir.dt.int16
    Alu = mybir.AluOpType
    Act = mybir.ActivationFunctionType

    caps, spg = host["caps"], host["spg"]
    albase, spbase = host["albase"], host["spbase"]
    L_AL, L_SP = host["L_AL"], host["L_SP"]
    runs = host["runs"]

    nc = bacc.Bacc("TRN2")

    # ---- DRAM tensors ----
    xT_d = nc.dram_tensor("xT", [128, NP], bft, kind="ExternalInput")
    xown_d = nc.dram_tensor("x_own", [OWN, D], fp32, kind="ExternalInput")
    Wp_d = nc.dram_tensor("Wp", [128, 136], bft, kind="ExternalInput")
    Wad_d = nc.dram_tensor("Wad", [128, 8], bft, kind="ExternalInput")
    xoT_d = nc.dram_tensor("x_ownT", [128, OWN], bft, kind="ExternalInput")
    I128_d = nc.dram_tensor("I128", [128, 128], bft, kind="ExternalInput")
    iota8_d = nc.dram_tensor("iota8", [128, BLK * SP_GPC], bft, kind="ExternalInput")
    W1_d = nc.dram_tensor("W1", [128, 256], bft, kind="ExternalInput")
    W2_d = nc.dram_tensor("W2", [256, 128], bft, kind="ExternalInput")
    b1c_d = nc.dram_tensor("b1col", [128, 2], fp32, kind="ExternalInput")
    gl_d = {}
    if not host["triv_bgat"]:
        gl_d["bgat"] = nc.dram_tensor("bgat_r", [128, 128], fp32, kind="ExternalInput")
    if not host["triv_bff2"]:
        gl_d["bff2"] = nc.dram_tensor("bff2_r", [128, 128], fp32, kind="ExternalInput")
    if not host["triv_gb1"]:
        gl_d["g1"] = nc.dram_tensor("g1_r", [128, 128], fp32, kind="ExternalInput")
        gl_d["b1"] = nc.dram_tensor("b1_r", [128, 128], fp32, kind="ExternalInput")
    if not host["triv_gb2"]:
        gl_d["g2"] = nc.dram_tensor("g2_r", [128, 128], fp32, kind="ExternalInput")
        gl_d["b2"] = nc.dram_tensor("b2_r", [128, 128], fp32, kind="ExternalInput")

    sd = {}
    for s, nm in ((0, "l"), (1, "h")):
        if L_AL[s]:
            sd[f"gidx_a{nm}"] = nc.dram_tensor(
                f"gidx_a{nm}", [128, L_AL[s] // 16], i16, kind="ExternalInput")
        if L_SP[s]:
            sd[f"gidx_s{nm}"] = nc.dram_tensor(
                f"gidx_s{nm}", [128, L_SP[s] // 16], i16, kind="ExternalInput")
            sd[f"aidx_s{nm}"] = nc.dram_tensor(
                f"aidx_s{nm}", [128, L_SP[s] // 16], i16, kind="ExternalInput")
            sd[f"dl_s{nm}"] = nc.dram_tensor(
                f"dl_s{nm}", [128, L_SP[s] // 128], bft, kind="ExternalInput")

    h_d = nc.dram_tensor("h_scratch", [HD_ROWS, 256], bft, kind="Internal")
    ad_d = nc.dram_tensor("adst_scratch", [OWN, 128], bft, kind="Internal")
    z_d = nc.dram_tensor("z", [OWN, D], fp32, kind="ExternalOutput")

    h_lo = h_d[0:32768, :]
    h_hi = h_d[32768:HD_ROWS, :]

    NT = NP // 128                    # 392 node tiles
    with tile.TileContext(nc) as tc:
        # ================= consts =================
        cpool = tc.alloc_tile_pool(name="consts", bufs=1)
        Wp_s = cpool.tile([128, 136], bft)
        nc.sync.dma_start(out=Wp_s[:], in_=Wp_d[:])
        Wad_s = cpool.tile([128, 8], bft)
        nc.sync.dma_start(out=Wad_s[:], in_=Wad_d[:])
        I128_s = cpool.tile([128, 128], bft)
        nc.sync.dma_start(out=I128_s[:], in_=I128_d[:])
        iota8_s = cpool.tile([128, BLK, SP_GPC], bft)
        nc.sync.dma_start(out=iota8_s[:], in_=iota8_d[:].rearrange(
            "p (n g) -> p n g", g=SP_GPC))
        W1_s = cpool.tile([128, 256], bft)
        nc.sync.dma_start(out=W1_s[:], in_=W1_d[:])
        W2_s = cpool.tile([256 // 2, 2, 128], bft)
        nc.sync.dma_start(out=W2_s[:],
                          in_=W2_d[:].rearrange("(k h) f -> h k f", k=2))
        b1c_s = cpool.tile([128, 2], fp32)
        nc.sync.dma_start(out=b1c_s[:], in_=b1c_d[:])
        gl_s = {}
        for k, dref in gl_d.items():
            gl_s[k] = cpool.tile([128, 128], fp32, tag=f"gl_{k}")
            nc.sync.dma_start(out=gl_s[k][:], in_=dref[:])
        eps_s = cpool.tile([128, 1], fp32)
        nc.vector.memset(eps_s[:], LN_EPS)
        ad_sb = cpool.tile([128, NBLK, 8], bft)

        # pad rows for h_d (row 0 and row NP+1): zeros, a_src cols = -200
        padr = cpool.tile([128, 256], bft)
        nc.vector.memset(padr[:], 0.0)
        nc.vector.memset(padr[:, 128:136], -200.0)
        nc.sync.dma_start(out=h_d[0:1, :], in_=padr[0:1, :])
        nc.sync.dma_start(out=h_d[NP + 1:NP + 2, :], in_=padr[0:1, :])

        # persistent phase-C tiles
        xo_all = cpool.tile([128, NBLK, 128], fp32)   # xo -> t1 -> t2 (reused)
        u_all = cpool.tile([128, NBLK, 128], fp32)
        mv1 = cpool.tile([128, NBLK, 2], fp32)
        mv2 = cpool.tile([128, NBLK, 2], fp32)
        sc1 = cpool.tile([128, NBLK], fp32)
        sc2 = cpool.tile([128, NBLK], fp32)
        nc.sync.dma_start(out=xo_all[:],
                          in_=xown_d[:].rearrange("(j n) d -> n j d", j=NBLK))

        # ================= phase A =================
        with tc.tile_pool(name="pA", bufs=4) as pA, \
             tc.tile_pool(name="psA", bufs=4, space="PSUM") as psA:
            GT = 3
            XB = 12
            xt = None
            for tg in range((NT + GT - 1) // GT):
                t0 = tg * GT
                ntl = min(GT, NT - t0)
                if t0 % XB == 0:
                    nxb = min(XB, NT - t0)
                    xt = pA.tile([128, XB * 128], bft, tag="xt")
                    nc.sync.dma_start(out=xt[:, :nxb * 128],
                                      in_=xT_d[:, t0 * 128:(t0 + nxb) * 128])
                ps = psA.tile([128, GT, 136], fp32, tag="psA")
                for j in range(ntl):
                    jo = (t0 % XB) + j
                    nc.tensor.matmul(ps[:, j, :],
                                     lhsT=xt[:, jo * 128:(jo + 1) * 128],
                                     rhs=Wp_s[:], start=True, stop=True)
                stage = pA.tile([128, GT, 256], bft, tag="stage")
                if tg % 2 == 0:
                    nc.scalar.activation(out=stage[:, :ntl, 0:136],
                                         in_=ps[:, :ntl, :], func=Act.Copy)
                else:
                    nc.vector.tensor_copy(out=stage[:, :ntl, 0:136],
                                          in_=ps[:, :ntl, :])
                nc.sync.dma_start(
                    out=h_d[t0 * 128 + 1:(t0 + ntl) * 128 + 1, :].rearrange(
                        "(j n) d -> n j d", j=ntl),
                    in_=stage[:, :ntl, :])

            # per-core a_dst of own nodes: x_ownT @ Wad
            GT2 = 7
            xoT = pA.tile([128, OWN], bft, tag="xoT")
            nc.sync.dma_start(out=xoT[:], in_=xoT_d[:])
            for t0 in range(0, NBLK, GT2):
                ps2 = psA.tile([128, GT2, 8], fp32, tag="psA2")
                for j in range(GT2):
                    t = t0 + j
                    nc.tensor.matmul(ps2[:, j, :],
                                     lhsT=xoT[:, t * 128:(t + 1) * 128],
                                     rhs=Wad_s[:], start=True, stop=True)
                nc.vector.tensor_copy(out=ad_sb[:, t0:t0 + GT2, :], in_=ps2[:])
                nc.sync.dma_start(
                    out=ad_d[t0 * 128:(t0 + GT2) * 128, 0:8].rearrange(
                        "(j n) d -> n j d", j=GT2),
                    in_=ad_sb[:, t0:t0 + GT2, :])

        tc.strict_bb_all_engine_barrier()

        dbg = _os.environ.get("GAT_DBG")
        if dbg in ("h", "ae"):
            with tc.tile_pool(name="dbg", bufs=2) as dp:
                for t in range(NBLK):
                    dt_ = dp.tile([128, 256], bft, tag="d")
                    nc.sync.dma_start(out=dt_[:],
                                      in_=h_d[1 + t * 128: 1 + (t + 1) * 128, :])
                    df = dp.tile([128, 128], fp32, tag="df")
                    if dbg == "h":
                        nc.vector.tensor_copy(out=df[:], in_=dt_[:, 0:128])
                    else:
                        nc.vector.memset(df[:], 0.0)
                        nc.vector.tensor_copy(out=df[:, 0:16], in_=dt_[:, 128:144])
                    nc.sync.dma_start(out=z_d[t * 128:(t + 1) * 128, :], in_=df[:])
            _finish_dbg = True
        else:
            _finish_dbg = False

        # ================= phase B =================
        pB = tc.alloc_tile_pool(name="pB", bufs=2)
        pBs = tc.alloc_tile_pool(name="pBsmall", bufs=4)
        psB = tc.alloc_tile_pool(name="psB", bufs=2, space="PSUM")
        pC = tc.alloc_tile_pool(name="pC", bufs=2)
        psC = tc.alloc_tile_pool(name="psC", bufs=2, space="PSUM")

        SCFG = {
            "AL": dict(gpc=AL_GPC, al=True, src=h_lo, gx="gidx_al", s=0),
            "AH": dict(gpc=AL_GPC, al=True, src=h_hi, gx="gidx_ah", s=1),
            "SL": dict(gpc=SP_GPC, al=False, src=h_lo, gx="gidx_sl",
                       ax="aidx_sl", dl="dl_sl", s=0),
            "SH": dict(gpc=SP_GPC, al=False, src=h_hi, gx="gidx_sh",
                       ax="aidx_sh", dl="dl_sh", s=1),
        }
        chunk_tiles = {st: {} for st in SCFG}

        def emit_chunk(st, k):
            ct = chunk_tiles[st]
            if k in ct:
                return ct[k]
            cfg = SCFG[st]
            gpc = cfg["gpc"]
            ch = gpc * 128
            gix = pBs.tile([128, ch // 16], i16, tag=f"gix{st}")
            nc.sync.dma_start(
                out=gix[:],
                in_=sd[cfg["gx"]][:, k * (ch // 16):(k + 1) * (ch // 16)])
            h_ch = pB.tile([128, gpc, 256], bft, tag=f"h{st}")
            nc.gpsimd.dma_gather(h_ch[:], cfg["src"], gix[:], ch, ch, 256,
                                 single_packet=False)
            eL = pBs.tile([128, gpc, 8], bft, tag=f"eL{st}")
            res = {}
            if cfg["al"]:
                for (g0, g1, b) in runs[cfg["s"]][k]:
                    sl = ad_sb[:, b, :]
                    ad_b = AP(sl.tensor, sl.offset,
                              [sl.ap[0], [0, g1 - g0], sl.ap[1]])
                    nc.vector.tensor_tensor(out=eL[:, g0:g1, :],
                                            in0=h_ch[:, g0:g1, 128:136],
                                            in1=ad_b, op=Alu.add)
            else:
                aix = pBs.tile([128, ch // 16], i16, tag=f"aix{st}")
                nc.sync.dma_start(
                    out=aix[:],
                    in_=sd[cfg["ax"]][:, k * (ch // 16):(k + 1) * (ch // 16)])
                adE = pB.tile([128, gpc, 128], bft, tag=f"adE{st}")
                nc.gpsimd.dma_gather(adE[:], ad_d[:], aix[:], ch, ch, 128,
                                     single_packet=False)
                nc.vector.tensor_tensor(out=eL[:], in0=h_ch[:, :, 128:136],
                                        in1=adE[:, :, 0:8], op=Alu.add)
                dlt = pBs.tile([128, gpc], bft, tag=f"dl{st}")
                nc.sync.dma_start(out=dlt[:],
                                  in_=sd[cfg["dl"]][:, k * gpc:(k + 1) * gpc])
                S2 = pB.tile([128, BLK, gpc], bft, tag=f"S2{st}")
                dsl = dlt[:]
                dl_b = AP(dsl.tensor, dsl.offset,
                          [dsl.ap[0], [0, BLK], dsl.ap[1]])
                nc.vector.tensor_tensor(out=S2[:], in0=dl_b,
                                        in1=iota8_s[:, :, 0:gpc],
                                        op=Alu.is_equal)
                res["S2"] = S2
            eL2 = pBs.tile([128, gpc, 8], bft, tag=f"eL2{st}")
            nc.vector.scalar_tensor_tensor(out=eL2[:], in0=eL[:], scalar=0.2,
                                           in1=eL[:], op0=Alu.mult, op1=Alu.max)
            msgp = pB.tile([128, gpc, 136], bft, tag=f"msgp{st}")
            nc.scalar.activation(out=msgp[:, :, 128:136], in_=eL2[:],
                                 func=Act.Exp)
            if bool(int(_os.environ.get("GAT_NO_PEXP", "0"))):
                nc.vector.tensor_tensor(
                    out=msgp[:, :, 0:128].rearrange("p g (h f) -> p g h f", f=F),
                    in0=h_ch[:, :, 0:128].rearrange("p g (h f) -> p g h f", f=F),
                    in1=msgp[:, :, 128:136].to_broadcast([128, gpc, 8, F]),
                    op=Alu.mult)
            else:
                nc.scalar.activation(
                    out=h_ch[:, :, 128:256].rearrange("p g (h f) -> p g h f", f=F),
                    in_=eL2[:].to_broadcast([128, gpc, 8, F]), func=Act.Exp)
                nc.vector.tensor_tensor(out=msgp[:, :, 0:128],
                                        in0=h_ch[:, :, 0:128],
                                        in1=h_ch[:, :, 128:256], op=Alu.mult)
            res["msgp"] = msgp
            ct[k] = res
            return res

        no_spill = bool(int(_os.environ.get("GAT_NO_SPILL", "0")))
        no_al = bool(int(_os.environ.get("GAT_NO_AL", "0")))
        no_c = bool(int(_os.environ.get("GAT_NO_C", "0")))
        if _finish_dbg or dbg in ("den", "gt"):
            no_c = True
        for b in range(NBLK if not _finish_dbg else 0):
            glist = []
            if not no_al:
                for s, st in ((0, "AL"), (1, "AH")):
                    for o in range(int(caps[b, s])):
                        glist.append((st, int(albase[b, s]) + o))
            if not no_spill:
                for s, st in ((0, "SL"), (1, "SH")):
                    for o in range(int(spg[b, s])):
                        glist.append((st, int(spbase[b, s]) + o))
            tot = len(glist)
            ps_blk = psB.tile([128, 136], fp32, tag="blk")
            for i, (st, gg) in enumerate(glist):
                cfg = SCFG[st]
                res = emit_chunk(st, gg // cfg["gpc"])
                gl = gg % cfg["gpc"]
                lhsT = I128_s[:] if cfg["al"] else res["S2"][:, :, gl]
                nc.tensor.matmul(ps_blk[:], lhsT=lhsT,
                                 rhs=res["msgp"][:, gl, :],
                                 start=(i == 0), stop=(i == tot - 1))
            # normalize: gt = agg/denom; t1 = x + gt (into xo_all)
            if dbg == "den":
                den = pBs.tile([128, 128], fp32, tag="den")
                nc.vector.memset(den[:], 0.0)
                nc.vector.tensor_copy(out=den[:, 0:8], in_=ps_blk[:, 128:136])
                nc.sync.dma_start(out=z_d[b * 128:(b + 1) * 128, :], in_=den[:])
                continue
            rec = pBs.tile([128, 8], fp32, tag="rec")
            nc.vector.reciprocal(out=rec[:], in_=ps_blk[:, 128:136])
            gt = pBs.tile([128, 128], fp32, tag="gt")
            nc.vector.tensor_tensor(
                out=gt[:].rearrange("p (h f) -> p h f", f=F),
                in0=ps_blk[:, 0:128].rearrange("p (h f) -> p h f", f=F),
                in1=rec[:].to_broadcast([128, 8, F]), op=Alu.mult)
            if not host["triv_bgat"]:
                nc.vector.tensor_tensor(out=gt[:], in0=gt[:],
                                        in1=gl_s["bgat"][:], op=Alu.add)
            if dbg == "gt":
                nc.sync.dma_start(out=z_d[b * 128:(b + 1) * 128, :], in_=gt[:])
                continue
            nc.vector.tensor_tensor(out=xo_all[:, b, :], in0=xo_all[:, b, :],
                                    in1=gt[:], op=Alu.add)

        # ================= phase C (batched sweep) =================
        t1_all = xo_all
        if no_c and not _finish_dbg and dbg not in ("den", "gt"):
            for b in range(NBLK):
                nc.sync.dma_start(out=z_d[b * 128:(b + 1) * 128, :],
                                  in_=t1_all[:, b, :])
        for b in range(NBLK if not no_c else 0):
            bst = pBs.tile([128, 6], fp32, tag="bst")
            nc.vector.bn_stats(out=bst[:], in_=t1_all[:, b, :])
            nc.vector.bn_aggr(out=mv1[:, b, :], in_=bst[:])
        if no_c:
            NBLK_C = 0
        else:
            NBLK_C = NBLK
            nc.scalar.activation(out=sc1[:], in_=mv1[:, :, 1], func=Act.Sqrt,
                                 bias=eps_s[:])
            nc.vector.reciprocal(out=sc1[:], in_=sc1[:])

        for b in range(NBLK_C):
            u = u_all[:, b, :]
            nc.vector.tensor_scalar(out=u, in0=t1_all[:, b, :],
                                    scalar1=mv1[:, b, 0:1], op0=Alu.subtract,
                                    scalar2=sc1[:, b:b + 1], op1=Alu.mult)
            if not host["triv_gb1"]:
                nc.vector.tensor_tensor(out=u, in0=u, in1=gl_s["g1"][:],
                                        op=Alu.mult)
                nc.vector.tensor_tensor(out=u, in0=u, in1=gl_s["b1"][:],
                                        op=Alu.add)
            u_bf = pC.tile([128, 128], bft, tag="ubf")
            nc.scalar.activation(out=u_bf[:], in_=u, func=Act.Copy)
            uT_ps = psC.tile([128, 128], bft, tag="uT")
            nc.tensor.transpose(uT_ps[:], in_=u_bf[:], identity=I128_s[:])
            uT = pC.tile([128, 128], bft, tag="uTs")
            nc.scalar.activation(out=uT[:], in_=uT_ps[:], func=Act.Copy)
            f1ps = psC.tile([128, 2, 128], fp32, tag="f1")
            for j in range(2):
                nc.tensor.matmul(f1ps[:, j, :], lhsT=W1_s[:, j * 128:(j + 1) * 128],
                                 rhs=uT[:], start=True, stop=True)
            r1 = pC.tile([128, 2, 128], bft, tag="r1")
            for j in range(2):
                nc.scalar.activation(out=r1[:, j, :], in_=f1ps[:, j, :],
                                     func=Act.Relu, bias=b1c_s[:, j:j + 1])
            zps = psC.tile([128, 128], fp32, tag="zp")
            for j in range(2):
                nc.tensor.matmul(zps[:], lhsT=r1[:, j, :], rhs=W2_s[:, j, :],
                                 start=(j == 0), stop=(j == 1))
            t2 = t1_all[:, b, :]
            nc.vector.tensor_tensor(out=t2, in0=u, in1=zps[:], op=Alu.add)
            if not host["triv_bff2"]:
                nc.vector.tensor_tensor(out=t2, in0=t2, in1=gl_s["bff2"][:],
                                        op=Alu.add)
            bst = pBs.tile([128, 6], fp32, tag="bst")
            nc.vector.bn_stats(out=bst[:], in_=t2)
            nc.vector.bn_aggr(out=mv2[:, b, :], in_=bst[:])
        if not no_c:
            nc.scalar.activation(out=sc2[:], in_=mv2[:, :, 1], func=Act.Sqrt,
                                 bias=eps_s[:])
            nc.vector.reciprocal(out=sc2[:], in_=sc2[:])

        for b in range(NBLK_C):
            zt = pC.tile([128, 128], fp32, tag="zt")
            nc.vector.tensor_scalar(out=zt[:], in0=t1_all[:, b, :],
                                    scalar1=mv2[:, b, 0:1], op0=Alu.subtract,
                                    scalar2=sc2[:, b:b + 1], op1=Alu.mult)
            if not host["triv_gb2"]:
                nc.vector.tensor_tensor(out=zt[:], in0=zt[:], in1=gl_s["g2"][:],
                                        op=Alu.mult)
                nc.vector.tensor_tensor(out=zt[:], in0=zt[:], in1=gl_s["b2"][:],
                                        op=Alu.add)
            nc.sync.dma_start(out=z_d[b * 128:(b + 1) * 128, :], in_=zt[:])

        for p in (psC, pC, psB, pBs, pB):
            p.release()
        cpool.release()

    nc.compile()
    return nc


def kernel(**inputs):
    import os
    from concourse.bass_utils import run_bass_kernel_spmd

    host = _build_host_data(inputs)
    nc = _build_program(host)

    in_maps = []
    for c in range(NCORES):
        m = {
            "xT": host["xT"],
            "x_own": host["x_own"][c],
            "x_ownT": host["x_ownT"][c],
            "Wp": host["Wp"], "Wad": host["Wad"],
            "I128": host["I128"], "iota8": host["iota8"],
            "W1": host["W1"], "W2": host["W2"], "b1col": host["b1col"],
        }
        if not host["triv_bgat"]:
            m["bgat_r"] = np.tile(host["bias_gat"].reshape(1, -1), (128, 1))
        if not host["triv_bff2"]:
            m["bff2_r"] = np.tile(host["b_ff2"].reshape(1, -1), (128, 1))
        if not host["triv_gb1"]:
            m["g1_r"] = np.tile(host["gamma1"].reshape(1, -1), (128, 1))
            m["b1_r"] = np.tile(host["beta1"].reshape(1, -1), (128, 1))
        if not host["triv_gb2"]:
            m["g2_r"] = np.tile(host["gamma2"].reshape(1, -1), (128, 1))
            m["b2_r"] = np.tile(host["beta2"].reshape(1, -1), (128, 1))
        for key in ("gidx_al", "gidx_ah", "gidx_sl", "gidx_sh",
                    "aidx_sl", "aidx_sh", "dl_sl", "dl_sh"):
            if key in host["per_core"][c]:
                m[key] = host["per_core"][c][key]
        in_maps.append(m)

    trace = bool(int(os.environ.get("GAT_TRACE", "0")))
    res = run_bass_kernel_spmd(nc, in_maps, core_ids=list(range(NCORES)),
                               trace=trace)
    if trace and res.exec_time_ns:
        print(f"HW exec time: {res.exec_time_ns} ns")
    if bool(int(os.environ.get("GAT_TIME", "0"))):
        try:
            from concourse.timeline_sim import TimelineSim
            ts = TimelineSim(nc)
            dur = ts.simulate()
            print(f"HW exec time: {dur:.0f} ns (cost-model timeline estimate)")
        except Exception as e:
            print("timeline sim failed:", e)

    out = np.zeros((N, D), np.float32)
    for c in range(NCORES):
        lo_n = OWN * c
        hi_n = min(OWN * (c + 1), N)
        out[lo_n:hi_n] = res.results[c]["z"][: hi_n - lo_n]
    return out


# revision 17
# speedup vs baseline: 1.3154x; 1.2664x over previous
"""Trainium2 Bass kernel for a GAT block (GATConv + LN + FFN + LN).

v2: partition-aligned destination scheme.

Per-core plan (identity node order; core c owns nodes [OWN*c, OWN*(c+1))):
  Phase A: hp = x @ [W | W@Asrc | W@Adst] for all 50176 nodes ([128,144] psum
           per tile); rows [h(128) | a_src(8) | pad] stored to h_d (512B bf16
           rows, +1 row offset; row 0 / row NP+1 are pad rows with
           a_src = -200 so padded gather slots contribute exp(...)=~0).
           a_dst for own nodes kept in SBUF (ad_sb [128, NBLK, 8] bf16) and
           also written to ad_d [OWN,128] for the spill path.
  Phase B: edges partitioned by dst-block (128 own nodes per block).
           ALIGNED streams (AL/AH = src row below/above 32767): granule g of
           block b holds, at partition p, the g-th in-edge of node p. One
           512B-row dma_gather per slot brings [h | a_src]; a_dst is read
           directly from ad_sb (lane == dst). p = exp(leaky(a_src+a_dst));
           pexp (p broadcast over F) is written by the Act engine into the
           pad half of the gathered rows so msg = h*pexp is a fully-packed
           bf16 DVE op (2x mode). Scatter = psum accumulate with lhsT=I.
           SPILL streams (SL/SH): per-(block,stream) overflow edges beyond
           the per-lane cap, dst-grouped; S built via is_equal with a
           middle-broadcast AP (2x mode), a_dst gathered from ad_d;
           scatter matmul with lhsT = S[:, :, g].
  Phase C: deferred, batched: per-block bn_stats/apply with the two Sqrt ops
           batched over all blocks (2 act-table loads total).
"""
import numpy as np
import ml_dtypes

N = 50000
NCORES = 8
OWN = 6272             # nodes per core (49 blocks of 128)
NP = OWN * NCORES      # 50176
BLK = 128
NBLK = OWN // BLK      # 49
H, F, D = 8, 16, 128
LN_EPS = 1e-5

LO_SPLIT = 32767       # src node < LO_SPLIT -> lo stream (h_d row = src+1)
HD_ROWS = 50432        # h_d rows: 0 pad_lo, 1..NP nodes, NP+1 pad_hi
PAD_HI_IDX = NP + 1 - 32768   # 17409
AL_GPC = 24            # granules per aligned chunk (3072 slots)
SP_GPC = 8             # granules per spill chunk (1024 slots)
KAL, KSP = 2.2, 4.7    # per-slot cost weights for cap optimization
PAD_DL = 200.0

bf16 = ml_dtypes.bfloat16


def _wrap16(idx):
    L = idx.shape[0]
    w = idx.reshape(L // 16, 16).T.astype(np.int16)
    return np.tile(w, (8, 1))                      # [128, L/16]


def _bfr(x):
    return np.ascontiguousarray(np.asarray(x, dtype=np.float32).astype(bf16))


def _build_host_data(inputs):
    x = np.asarray(inputs["x"], np.float32)
    W = np.asarray(inputs["W_gat"], np.float32)
    att_src = np.asarray(inputs["att_src"], np.float32)
    att_dst = np.asarray(inputs["att_dst"], np.float32)
    ei = np.asarray(inputs["edge_index"])

    src = ei[0].astype(np.int64)
    dst = ei[1].astype(np.int64)
    loops = np.arange(N, dtype=np.int64)
    src = np.concatenate([src, loops])
    dst = np.concatenate([dst, loops])

    # ---- per-core edge tables (identity node order) ----
    deg = np.zeros((NCORES, NBLK, BLK, 2), dtype=np.int32)
    core_e = []
    for c in range(NCORES):
        m = (dst >= OWN * c) & (dst < min(OWN * (c + 1), N))
        e_src = src[m]
        d_l = dst[m] - OWN * c
        b = d_l >> 7
        lane = d_l & 127
        s = (e_src >= LO_SPLIT).astype(np.int64)
        np.add.at(deg, (c, b, lane, s), 1)
        core_e.append((e_src, d_l, b, lane, s))

    # ---- shared caps per (block, stream); spill granule profile ----
    caps = np.zeros((NBLK, 2), dtype=np.int64)
    spg = np.zeros((NBLK, 2), dtype=np.int64)
    for b in range(NBLK):
        for s in range(2):
            d = deg[:, b, :, s]                       # [cores, 128]
            dmax = int(d.max())
            caps_r = np.arange(dmax + 1)
            spill = np.maximum(d[:, :, None] - caps_r[None, None, :], 0
                               ).sum(axis=1).max(axis=0)     # [dmax+1]
            g_sp = -(-spill // 128)
            cost = 128 * caps_r * KAL + 128 * g_sp * KSP
            k = int(np.argmin(cost))
            caps[b, s] = k
            spg[b, s] = g_sp[k]

    albase = np.zeros((NBLK, 2), dtype=np.int64)
    spbase = np.zeros((NBLK, 2), dtype=np.int64)
    albase[1:] = np.cumsum(caps[:-1], axis=0)
    spbase[1:] = np.cumsum(spg[:-1], axis=0)
    G_AL = [int(caps[:, s].sum()) for s in range(2)]
    G_SP = [int(spg[:, s].sum()) for s in range(2)]
    # pad granule counts to chunk multiples
    G_ALp = [-(-g // AL_GPC) * AL_GPC if g else 0 for g in G_AL]
    G_SPp = [-(-g // SP_GPC) * SP_GPC if g else 0 for g in G_SP]
    L_AL = [g * 128 for g in G_ALp]
    L_SP = [g * 128 for g in G_SPp]

    # block-of-granule maps (shared); pad granules -> block 0
    blk_of_g = []
    for s in range(2):
        bg = np.repeat(np.arange(NBLK), caps[:, s])
        bg = np.concatenate([bg, np.zeros(G_ALp[s] - len(bg), dtype=np.int64)])
        blk_of_g.append(bg)
    # runs per aligned chunk: list of (g0_local, g1_local, block)
    runs = []
    for s in range(2):
        rs = []
        bg = blk_of_g[s]
        for k in range(G_ALp[s] // AL_GPC):
            seg = bg[k * AL_GPC:(k + 1) * AL_GPC]
            r = []
            i = 0
            while i < AL_GPC:
                j = i
                while j < AL_GPC and seg[j] == seg[i]:
                    j += 1
                r.append((i, j, int(seg[i])))
                i = j
            rs.append(r)
        runs.append(rs)

    # ---- per-core slot data ----
    per_core = []
    for c in range(NCORES):
        e_src, d_l, b, lane, s = core_e[c]
        key = (b * 128 + lane) * 2 + s
        order = np.argsort(key, kind="stable")
        ks = key[order]
        chg = np.r_[True, ks[1:] != ks[:-1]] if len(ks) else np.array([], bool)
        grp_id = np.cumsum(chg) - 1 if len(ks) else ks
        starts = np.flatnonzero(chg)
        occ = np.arange(len(ks)) - starts[grp_id] if len(ks) else ks
        so, bo, lo_, eo = s[order], b[order], lane[order], e_src[order]
        dlo = d_l[order]
        cap_e = caps[bo, so]
        al_m = occ < cap_e

        enc = np.where(so == 0, eo + 1, eo + 1 - 32768).astype(np.int64)

        gidx_al = [np.zeros(L_AL[0], dtype=np.int64),
                   np.full(L_AL[1], PAD_HI_IDX, dtype=np.int64)]
        for s_ in range(2):
            mm = al_m & (so == s_)
            g = albase[bo[mm], s_] + occ[mm]
            slot = g * 128 + lo_[mm]
            gidx_al[s_][slot] = enc[mm]

        gidx_sp = [np.zeros(L_SP[0], dtype=np.int64),
                   np.full(L_SP[1], PAD_HI_IDX, dtype=np.int64)]
        dl_sp = [np.full(L_SP[s_], PAD_DL, dtype=np.float32) for s_ in range(2)]
        aidx_sp = [np.zeros(L_SP[s_], dtype=np.int64) for s_ in range(2)]
        sp_m = ~al_m
        key2 = bo[sp_m] * 2 + so[sp_m]
        order2 = np.argsort(key2, kind="stable")
        k2 = key2[order2]
        chg2 = np.r_[True, k2[1:] != k2[:-1]] if len(k2) else np.array([], bool)
        gid2 = np.cumsum(chg2) - 1 if len(k2) else k2
        st2 = np.flatnonzero(chg2)
        rank2 = np.arange(len(k2)) - st2[gid2] if len(k2) else k2
        b2 = bo[sp_m][order2]
        s2 = so[sp_m][order2]
        e2 = enc[sp_m][order2]
        lane2 = lo_[sp_m][order2]
        dl2 = dlo[sp_m][order2]
        for s_ in range(2):
            mm = s2 == s_
            slot = spbase[b2[mm], s_] * 128 + rank2[mm]
            gidx_sp[s_][slot] = e2[mm]
            dl_sp[s_][slot] = lane2[mm].astype(np.float32)
            aidx_sp[s_][slot] = dl2[mm]

        ent = {}
        for s_, nm in ((0, "l"), (1, "h")):
            if L_AL[s_]:
                ent[f"gidx_a{nm}"] = _wrap16(gidx_al[s_])
            if L_SP[s_]:
                ent[f"gidx_s{nm}"] = _wrap16(gidx_sp[s_])
                ent[f"aidx_s{nm}"] = _wrap16(aidx_sp[s_])
                ent[f"dl_s{nm}"] = np.ascontiguousarray(
                    dl_sp[s_].astype(bf16).reshape(-1, 128).T)
        per_core.append(ent)

    # ---- weights / constants ----
    Asrc = np.zeros((D, H), np.float32)
    Adst = np.zeros((D, H), np.float32)
    for h in range(H):
        Asrc[h * F:(h + 1) * F, h] = att_src[h]
        Adst[h * F:(h + 1) * F, h] = att_dst[h]
    Wp = _bfr(np.concatenate([W, W @ Asrc], axis=1))             # [128,136]
    Wad = _bfr(W @ Adst)                                         # [128,8]
    I128 = _bfr(np.eye(128, dtype=np.float32))
    iota8 = _bfr(np.tile(np.arange(BLK, dtype=np.float32)[None, :, None],
                         (128, 1, SP_GPC)).reshape(128, BLK * SP_GPC))

    xp = np.zeros((NP, D), np.float32)
    xp[:N] = x
    xT = np.ascontiguousarray(xp.T.astype(bf16))                 # [128, NP]
    x_own = [np.ascontiguousarray(xp[OWN * c: OWN * (c + 1)]) for c in range(NCORES)]
    x_ownT = [np.ascontiguousarray(xp[OWN * c: OWN * (c + 1)].T.astype(bf16))
              for c in range(NCORES)]

    host = {
        "caps": caps, "spg": spg, "albase": albase, "spbase": spbase,
        "G_ALp": G_ALp, "G_SPp": G_SPp, "L_AL": L_AL, "L_SP": L_SP,
        "runs": runs, "per_core": per_core,
        "xT": xT, "x_own": x_own, "x_ownT": x_ownT, "Wp": Wp, "Wad": Wad,
        "I128": I128, "iota8": iota8,
        "W1": _bfr(np.asarray(inputs["w_ff1"], np.float32)),     # [128,256]
        "W2": _bfr(np.asarray(inputs["w_ff2"], np.float32)),     # [256,128]
        "b1col": np.ascontiguousarray(
            np.asarray(inputs["b_ff1"], np.float32).reshape(2, 128).T),  # [128,2]
    }
    host["bias_gat"] = np.asarray(inputs["bias_gat"], np.float32)
    host["b_ff2"] = np.asarray(inputs["b_ff2"], np.float32)
    for nm in ("gamma1", "beta1", "gamma2", "beta2"):
        host[nm] = np.asarray(inputs[nm], np.float32)
    host["triv_gb1"] = bool(np.all(host["gamma1"] == 1) and np.all(host["beta1"] == 0))
    host["triv_gb2"] = bool(np.all(host["gamma2"] == 1) and np.all(host["beta2"] == 0))
    host["triv_bgat"] = bool(np.all(host["bias_gat"] == 0))
    host["triv_bff2"] = bool(np.all(host["b_ff2"] == 0))
    return host


def _build_program(host):
    import os as _os
    import concourse.bacc as bacc
    import concourse.mybir as mybir
    import concourse.tile as tile
    from concourse.bass import AP

    fp32 = mybir.dt.float32
    bft = mybir.dt.bfloat16
    i16 = mybir.dt.int16
    Alu = mybir.AluOpType
    Act = mybir.ActivationFunctionType

    caps, spg = host["caps"], host["spg"]
    albase, spbase = host["albase"], host["spbase"]
    L_AL, L_SP = host["L_AL"], host["L_SP"]
    runs = host["runs"]

    nc = bacc.Bacc("TRN2")

    # ---- DRAM tensors ----
    xT_d = nc.dram_tensor("xT", [128, NP], bft, kind="ExternalInput")
    xown_d = nc.dram_tensor("x_own", [OWN, D], fp32, kind="ExternalInput")
    Wp_d = nc.dram_tensor("Wp", [128, 136], bft, kind="ExternalInput")
    Wad_d = nc.dram_tensor("Wad", [128, 8], bft, kind="ExternalInput")
    xoT_d = nc.dram_tensor("x_ownT", [128, OWN], bft, kind="ExternalInput")
    I128_d = nc.dram_tensor("I128", [128, 128], bft, kind="ExternalInput")
    iota8_d = nc.dram_tensor("iota8", [128, BLK * SP_GPC], bft, kind="ExternalInput")
    W1_d = nc.dram_tensor("W1", [128, 256], bft, kind="ExternalInput")
    W2_d = nc.dram_tensor("W2", [256, 128], bft, kind="ExternalInput")
    b1c_d = nc.dram_tensor("b1col", [128, 2], fp32, kind="ExternalInput")
    gl_d = {}
    if not host["triv_bgat"]:
        gl_d["bgat"] = nc.dram_tensor("bgat_r", [128, 128], fp32, kind="ExternalInput")
    if not host["triv_bff2"]:
        gl_d["bff2"] = nc.dram_tensor("bff2_r", [128, 128], fp32, kind="ExternalInput")
    if not host["triv_gb1"]:
        gl_d["g1"] = nc.dram_tensor("g1_r", [128, 128], fp32, kind="ExternalInput")
        gl_d["b1"] = nc.dram_tensor("b1_r", [128, 128], fp32, kind="ExternalInput")
    if not host["triv_gb2"]:
        gl_d["g2"] = nc.dram_tensor("g2_r", [128, 128], fp32, kind="ExternalInput")
        gl_d["b2"] = nc.dram_tensor("b2_r", [128, 128], fp32, kind="ExternalInput")

    sd = {}
    for s, nm in ((0, "l"), (1, "h")):
        if L_AL[s]:
            sd[f"gidx_a{nm}"] = nc.dram_tensor(
                f"gidx_a{nm}", [128, L_AL[s] // 16], i16, kind="ExternalInput")
        if L_SP[s]:
            sd[f"gidx_s{nm}"] = nc.dram_tensor(
                f"gidx_s{nm}", [128, L_SP[s] // 16], i16, kind="ExternalInput")
            sd[f"aidx_s{nm}"] = nc.dram_tensor(
                f"aidx_s{nm}", [128, L_SP[s] // 16], i16, kind="ExternalInput")
            sd[f"dl_s{nm}"] = nc.dram_tensor(
                f"dl_s{nm}", [128, L_SP[s] // 128], bft, kind="ExternalInput")

    h_d = nc.dram_tensor("h_scratch", [HD_ROWS, 256], bft, kind="Internal")
    ad_d = nc.dram_tensor("adst_scratch", [OWN, 128], bft, kind="Internal")
    z_d = nc.dram_tensor("z", [OWN, D], fp32, kind="ExternalOutput")

    h_lo = h_d[0:32768, :]
    h_hi = h_d[32768:HD_ROWS, :]

    NT = NP // 128                    # 392 node tiles
    with tile.TileContext(nc) as tc:
        # ================= consts =================
        cpool = tc.alloc_tile_pool(name="consts", bufs=1)
        Wp_s = cpool.tile([128, 136], bft)
        nc.sync.dma_start(out=Wp_s[:], in_=Wp_d[:])
        Wad_s = cpool.tile([128, 8], bft)
        nc.sync.dma_start(out=Wad_s[:], in_=Wad_d[:])
        I128_s = cpool.tile([128, 128], bft)
        nc.sync.dma_start(out=I128_s[:], in_=I128_d[:])
        iota8_s = cpool.tile([128, BLK, SP_GPC], bft)
        nc.sync.dma_start(out=iota8_s[:], in_=iota8_d[:].rearrange(
            "p (n g) -> p n g", g=SP_GPC))
        W1_s = cpool.tile([128, 256], bft)
        nc.sync.dma_start(out=W1_s[:], in_=W1_d[:])
        W2_s = cpool.tile([256 // 2, 2, 128], bft)
        nc.sync.dma_start(out=W2_s[:],
                          in_=W2_d[:].rearrange("(k h) f -> h k f", k=2))
        b1c_s = cpool.tile([128, 2], fp32)
        nc.sync.dma_start(out=b1c_s[:], in_=b1c_d[:])
        gl_s = {}
        for k, dref in gl_d.items():
            gl_s[k] = cpool.tile([128, 128], fp32, tag=f"gl_{k}")
            nc.sync.dma_start(out=gl_s[k][:], in_=dref[:])
        eps_s = cpool.tile([128, 1], fp32)
        nc.vector.memset(eps_s[:], LN_EPS)
        ad_sb = cpool.tile([128, NBLK, 8], bft)

        # pad rows for h_d (row 0 and row NP+1): zeros, a_src cols = -200
        padr = cpool.tile([128, 256], bft)
        nc.vector.memset(padr[:], 0.0)
        nc.vector.memset(padr[:, 128:136], -200.0)
        nc.sync.dma_start(out=h_d[0:1, :], in_=padr[0:1, :])
        nc.sync.dma_start(out=h_d[NP + 1:NP + 2, :], in_=padr[0:1, :])

        # persistent phase-C tiles
        xo_all = cpool.tile([128, NBLK, 128], fp32)   # xo -> t1 -> t2 (reused)
        u_all = cpool.tile([128, NBLK, 128], fp32)
        mv1 = cpool.tile([128, NBLK, 2], fp32)
        mv2 = cpool.tile([128, NBLK, 2], fp32)
        sc1 = cpool.tile([128, NBLK], fp32)
        sc2 = cpool.tile([128, NBLK], fp32)
        nc.sync.dma_start(out=xo_all[:],
                          in_=xown_d[:].rearrange("(j n) d -> n j d", j=NBLK))

        # ================= phase A =================
        with tc.tile_pool(name="pA", bufs=4) as pA, \
             tc.tile_pool(name="psA", bufs=4, space="PSUM") as psA:
            GT = 3
            SGT = 6
            XB = 12
            xt = None
            stage = None
            for tg in range((NT + GT - 1) // GT):
                t0 = tg * GT
                ntl = min(GT, NT - t0)
                if t0 % XB == 0:
                    nxb = min(XB, NT - t0)
                    xt = pA.tile([128, XB * 128], bft, tag="xt")
                    nc.scalar.dma_start(out=xt[:, :nxb * 128],
                                        in_=xT_d[:, t0 * 128:(t0 + nxb) * 128])
                ps = psA.tile([128, GT, 136], fp32, tag="psA")
                for j in range(ntl):
                    jo = (t0 % XB) + j
                    nc.tensor.matmul(ps[:, j, :],
                                     lhsT=xt[:, jo * 128:(jo + 1) * 128],
                                     rhs=Wp_s[:], start=True, stop=True)
                so = t0 % SGT
                if so == 0:
                    stage = pA.tile([128, SGT, 256], bft, tag="stage")
                if tg % 2 == 0:
                    nc.scalar.activation(out=stage[:, so:so + ntl, 0:136],
                                         in_=ps[:, :ntl, :], func=Act.Copy)
                else:
                    nc.vector.tensor_copy(out=stage[:, so:so + ntl, 0:136],
                                          in_=ps[:, :ntl, :])
                if so + ntl == SGT or t0 + ntl == NT:
                    s0 = t0 - so
                    nst = so + ntl
                    nc.sync.dma_start(
                        out=h_d[s0 * 128 + 1:(s0 + nst) * 128 + 1, :].rearrange(
                            "(j n) d -> n j d", j=nst),
                        in_=stage[:, :nst, :])

            # per-core a_dst of own nodes: x_ownT @ Wad
            GT2 = 7
            xoT = pA.tile([128, OWN], bft, tag="xoT")
            nc.sync.dma_start(out=xoT[:], in_=xoT_d[:])
            for t0 in range(0, NBLK, GT2):
                ps2 = psA.tile([128, GT2, 8], fp32, tag="psA2")
                for j in range(GT2):
                    t = t0 + j
                    nc.tensor.matmul(ps2[:, j, :],
                                     lhsT=xoT[:, t * 128:(t + 1) * 128],
                                     rhs=Wad_s[:], start=True, stop=True)
                nc.vector.tensor_copy(out=ad_sb[:, t0:t0 + GT2, :], in_=ps2[:])
                nc.sync.dma_start(
                    out=ad_d[t0 * 128:(t0 + GT2) * 128, 0:8].rearrange(
                        "(j n) d -> n j d", j=GT2),
                    in_=ad_sb[:, t0:t0 + GT2, :])

        tc.strict_bb_all_engine_barrier()

        dbg = _os.environ.get("GAT_DBG")
        if dbg in ("h", "ae"):
            with tc.tile_pool(name="dbg", bufs=2) as dp:
                for t in range(NBLK):
                    dt_ = dp.tile([128, 256], bft, tag="d")
                    nc.sync.dma_start(out=dt_[:],
                                      in_=h_d[1 + t * 128: 1 + (t + 1) * 128, :])
                    df = dp.tile([128, 128], fp32, tag="df")
                    if dbg == "h":
                        nc.vector.tensor_copy(out=df[:], in_=dt_[:, 0:128])
                    else:
                        nc.vector.memset(df[:], 0.0)
                        nc.vector.tensor_copy(out=df[:, 0:16], in_=dt_[:, 128:144])
                    nc.sync.dma_start(out=z_d[t * 128:(t + 1) * 128, :], in_=df[:])
            _finish_dbg = True
        else:
            _finish_dbg = False

        # ================= phase B =================
        pB = tc.alloc_tile_pool(name="pB", bufs=2)
        pBs = tc.alloc_tile_pool(name="pBsmall", bufs=4)
        psB = tc.alloc_tile_pool(name="psB", bufs=2, space="PSUM")
        pC = tc.alloc_tile_pool(name="pC", bufs=2)
        psC = tc.alloc_tile_pool(name="psC", bufs=2, space="PSUM")

        SCFG = {
            "AL": dict(gpc=AL_GPC, al=True, src=h_lo, gx="gidx_al", s=0),
            "AH": dict(gpc=AL_GPC, al=True, src=h_hi, gx="gidx_ah", s=1),
            "SL": dict(gpc=SP_GPC, al=False, src=h_lo, gx="gidx_sl",
                       ax="aidx_sl", dl="dl_sl", s=0),
            "SH": dict(gpc=SP_GPC, al=False, src=h_hi, gx="gidx_sh",
                       ax="aidx_sh", dl="dl_sh", s=1),
        }
        chunk_tiles = {st: {} for st in SCFG}

        def emit_chunk(st, k):
            ct = chunk_tiles[st]
            if k in ct:
                return ct[k]
            cfg = SCFG[st]
            gpc = cfg["gpc"]
            ch = gpc * 128
            nch = sd[cfg["gx"]].shape[1] // (ch // 16)   # total chunks
            k4 = k // 4
            gx4 = cfg.setdefault("_gx4", {})
            if k4 not in gx4:
                c0 = k4 * 4 * (ch // 16)
                c1 = min((k4 + 1) * 4 * (ch // 16), sd[cfg["gx"]].shape[1])
                t4 = pBs.tile([128, 4 * (ch // 16)], i16, tag=f"gix{st}")
                nc.scalar.dma_start(out=t4[:, :c1 - c0],
                                    in_=sd[cfg["gx"]][:, c0:c1])
                gx4[k4] = t4
            gix = gx4[k4][:, (k % 4) * (ch // 16):(k % 4 + 1) * (ch // 16)]
            h_ch = pB.tile([128, gpc, 256], bft, tag=f"h{st}")
            nc.gpsimd.dma_gather(h_ch[:], cfg["src"], gix, ch, ch, 256,
                                 single_packet=False)
            eL = pBs.tile([128, gpc, 8], bft, tag=f"eL{st}")
            res = {}
            if cfg["al"]:
                for (g0, g1, b) in runs[cfg["s"]][k]:
                    sl = ad_sb[:, b, :]
                    ad_b = AP(sl.tensor, sl.offset,
                              [sl.ap[0], [0, g1 - g0], sl.ap[1]])
                    nc.vector.tensor_tensor(out=eL[:, g0:g1, :],
                                            in0=h_ch[:, g0:g1, 128:136],
                                            in1=ad_b, op=Alu.add)
            else:
                ax4 = cfg.setdefault("_ax4", {})
                if k4 not in ax4:
                    c0 = k4 * 4 * (ch // 16)
                    c1 = min((k4 + 1) * 4 * (ch // 16), sd[cfg["ax"]].shape[1])
                    t4 = pBs.tile([128, 4 * (ch // 16)], i16, tag=f"aix{st}")
                    nc.scalar.dma_start(out=t4[:, :c1 - c0],
                                        in_=sd[cfg["ax"]][:, c0:c1])
                    ax4[k4] = t4
                aix = ax4[k4][:, (k % 4) * (ch // 16):(k % 4 + 1) * (ch // 16)]
                adE = pB.tile([128, gpc, 128], bft, tag=f"adE{st}")
                nc.gpsimd.dma_gather(adE[:], ad_d[:], aix, ch, ch, 128,
                                     single_packet=False)
                nc.vector.tensor_tensor(out=eL[:], in0=h_ch[:, :, 128:136],
                                        in1=adE[:, :, 0:8], op=Alu.add)
                dl4 = cfg.setdefault("_dl4", {})
                if k4 not in dl4:
                    c0 = k4 * 4 * gpc
                    c1 = min((k4 + 1) * 4 * gpc, sd[cfg["dl"]].shape[1])
                    t4 = pBs.tile([128, 4 * gpc], bft, tag=f"dl{st}")
                    nc.scalar.dma_start(out=t4[:, :c1 - c0],
                                        in_=sd[cfg["dl"]][:, c0:c1])
                    dl4[k4] = t4
                dlt = dl4[k4][:, (k % 4) * gpc:(k % 4 + 1) * gpc]
                S2 = pB.tile([128, BLK, gpc], bft, tag=f"S2{st}")
                dsl = dlt
                dl_b = AP(dsl.tensor, dsl.offset,
                          [dsl.ap[0], [0, BLK], dsl.ap[1]])
                nc.vector.tensor_tensor(out=S2[:], in0=dl_b,
                                        in1=iota8_s[:, :, 0:gpc],
                                        op=Alu.is_equal)
                res["S2"] = S2
            eL2 = pBs.tile([128, gpc, 8], bft, tag=f"eL2{st}")
            nc.vector.scalar_tensor_tensor(out=eL2[:], in0=eL[:], scalar=0.2,
                                           in1=eL[:], op0=Alu.mult, op1=Alu.max)
            msgp = pB.tile([128, gpc, 136], bft, tag=f"msgp{st}")
            nc.scalar.activation(out=msgp[:, :, 128:136], in_=eL2[:],
                                 func=Act.Exp)
            if bool(int(_os.environ.get("GAT_NO_PEXP", "0"))):
                nc.vector.tensor_tensor(
                    out=msgp[:, :, 0:128].rearrange("p g (h f) -> p g h f", f=F),
                    in0=h_ch[:, :, 0:128].rearrange("p g (h f) -> p g h f", f=F),
                    in1=msgp[:, :, 128:136].to_broadcast([128, gpc, 8, F]),
                    op=Alu.mult)
            else:
                nc.scalar.activation(
                    out=h_ch[:, :, 128:256].rearrange("p g (h f) -> p g h f", f=F),
                    in_=eL2[:].to_broadcast([128, gpc, 8, F]), func=Act.Exp)
                nc.vector.tensor_tensor(out=msgp[:, :, 0:128],
                                        in0=h_ch[:, :, 0:128],
                                        in1=h_ch[:, :, 128:256], op=Alu.mult)
            res["msgp"] = msgp
            ct[k] = res
            return res

        GRPS = [0, 13, 25, 37, NBLK]
        GRP_END = {GRPS[i + 1]: GRPS[i] for i in range(len(GRPS) - 1)}
        t1_all = xo_all

        def emit_group_c(b0, b1):
            nb = b1 - b0
            nc.scalar.activation(out=sc1[:, b0:b1], in_=mv1[:, b0:b1, 1],
                                 func=Act.Sqrt, bias=eps_s[:])
            nc.vector.reciprocal(out=sc1[:, b0:b1], in_=sc1[:, b0:b1])
            for b in range(b0, b1):
                u = u_all[:, b, :]
                nc.vector.tensor_scalar(out=u, in0=t1_all[:, b, :],
                                        scalar1=mv1[:, b, 0:1], op0=Alu.subtract,
                                        scalar2=sc1[:, b:b + 1], op1=Alu.mult)
                if not host["triv_gb1"]:
                    nc.vector.tensor_tensor(out=u, in0=u, in1=gl_s["g1"][:],
                                            op=Alu.mult)
                    nc.vector.tensor_tensor(out=u, in0=u, in1=gl_s["b1"][:],
                                            op=Alu.add)
                u_bf = pC.tile([128, 128], bft, tag="ubf")
                nc.gpsimd.tensor_copy(out=u_bf[:], in_=u)
                uT_ps = psC.tile([128, 128], bft, tag="uT")
                nc.tensor.transpose(uT_ps[:], in_=u_bf[:], identity=I128_s[:])
                uT = pC.tile([128, 128], bft, tag="uTs")
                nc.scalar.activation(out=uT[:], in_=uT_ps[:], func=Act.Copy)
                f1ps = psC.tile([128, 2, 128], fp32, tag="f1")
                for j in range(2):
                    nc.tensor.matmul(f1ps[:, j, :],
                                     lhsT=W1_s[:, j * 128:(j + 1) * 128],
                                     rhs=uT[:], start=True, stop=True)
                r1 = pC.tile([128, 2, 128], bft, tag="r1")
                nc.vector.tensor_scalar(out=r1[:, 0, :], in0=f1ps[:, 0, :],
                                        scalar1=b1c_s[:, 0:1], op0=Alu.add,
                                        scalar2=0.0, op1=Alu.max)
                nc.scalar.activation(out=r1[:, 1, :], in_=f1ps[:, 1, :],
                                     func=Act.Relu, bias=b1c_s[:, 1:2])
                zps = psC.tile([128, 128], fp32, tag="zp")
                for j in range(2):
                    nc.tensor.matmul(zps[:], lhsT=r1[:, j, :], rhs=W2_s[:, j, :],
                                     start=(j == 0), stop=(j == 1))
                t2 = t1_all[:, b, :]
                nc.vector.tensor_tensor(out=t2, in0=u, in1=zps[:], op=Alu.add)
                if not host["triv_bff2"]:
                    nc.vector.tensor_tensor(out=t2, in0=t2, in1=gl_s["bff2"][:],
                                            op=Alu.add)
                bst = pBs.tile([128, 6], fp32, tag="bst")
                nc.vector.bn_stats(out=bst[:], in_=t2)
                nc.vector.bn_aggr(out=mv2[:, b, :], in_=bst[:])
            nc.scalar.activation(out=sc2[:, b0:b1], in_=mv2[:, b0:b1, 1],
                                 func=Act.Sqrt, bias=eps_s[:])
            nc.vector.reciprocal(out=sc2[:, b0:b1], in_=sc2[:, b0:b1])
            for b in range(b0, b1):
                zt = u_all[:, b, :]
                nc.vector.tensor_scalar(out=zt, in0=t1_all[:, b, :],
                                        scalar1=mv2[:, b, 0:1], op0=Alu.subtract,
                                        scalar2=sc2[:, b:b + 1], op1=Alu.mult)
                if not host["triv_gb2"]:
                    nc.vector.tensor_tensor(out=zt, in0=zt, in1=gl_s["g2"][:],
                                            op=Alu.mult)
                    nc.vector.tensor_tensor(out=zt, in0=zt, in1=gl_s["b2"][:],
                                            op=Alu.add)
            nc.scalar.dma_start(
                out=z_d[b0 * 128:b1 * 128, :].rearrange("(j n) d -> n j d", j=nb),
                in_=u_all[:, b0:b1, :])

        no_spill = bool(int(_os.environ.get("GAT_NO_SPILL", "0")))
        no_al = bool(int(_os.environ.get("GAT_NO_AL", "0")))
        no_c = bool(int(_os.environ.get("GAT_NO_C", "0")))
        if _finish_dbg or dbg in ("den", "gt"):
            no_c = True
        for b in range(NBLK if not _finish_dbg else 0):
            glist = []
            if not no_al:
                for s, st in ((0, "AL"), (1, "AH")):
                    for o in range(int(caps[b, s])):
                        glist.append((st, int(albase[b, s]) + o))
            if not no_spill:
                for s, st in ((0, "SL"), (1, "SH")):
                    for o in range(int(spg[b, s])):
                        glist.append((st, int(spbase[b, s]) + o))
            tot = len(glist)
            ps_blk = psB.tile([128, 136], fp32, tag="blk")
            for i, (st, gg) in enumerate(glist):
                cfg = SCFG[st]
                res = emit_chunk(st, gg // cfg["gpc"])
                gl = gg % cfg["gpc"]
                lhsT = I128_s[:] if cfg["al"] else res["S2"][:, :, gl]
                nc.tensor.matmul(ps_blk[:], lhsT=lhsT,
                                 rhs=res["msgp"][:, gl, :],
                                 start=(i == 0), stop=(i == tot - 1))
            # normalize: gt = agg/denom; t1 = x + gt (into xo_all)
            if dbg == "den":
                den = pBs.tile([128, 128], fp32, tag="den")
                nc.vector.memset(den[:], 0.0)
                nc.vector.tensor_copy(out=den[:, 0:8], in_=ps_blk[:, 128:136])
                nc.sync.dma_start(out=z_d[b * 128:(b + 1) * 128, :], in_=den[:])
                continue
            rec = pBs.tile([128, 8], fp32, tag="rec")
            nc.vector.reciprocal(out=rec[:], in_=ps_blk[:, 128:136])
            gt = pBs.tile([128, 128], fp32, tag="gt")
            nc.vector.tensor_tensor(
                out=gt[:].rearrange("p (h f) -> p h f", f=F),
                in0=ps_blk[:, 0:128].rearrange("p (h f) -> p h f", f=F),
                in1=rec[:].to_broadcast([128, 8, F]), op=Alu.mult)
            if not host["triv_bgat"]:
                nc.vector.tensor_tensor(out=gt[:], in0=gt[:],
                                        in1=gl_s["bgat"][:], op=Alu.add)
            if dbg == "gt":
                nc.sync.dma_start(out=z_d[b * 128:(b + 1) * 128, :], in_=gt[:])
                continue
            nc.vector.tensor_tensor(out=xo_all[:, b, :], in0=xo_all[:, b, :],
                                    in1=gt[:], op=Alu.add)
            if not no_c:
                bst = pBs.tile([128, 6], fp32, tag="bst")
                nc.vector.bn_stats(out=bst[:], in_=xo_all[:, b, :])
                nc.vector.bn_aggr(out=mv1[:, b, :], in_=bst[:])
                if b + 1 in GRP_END:
                    emit_group_c(GRP_END[b + 1], b + 1)

        # ================= phase C emitted group-wise inside the block loop ====
        if no_c and dbg not in ("den", "gt"):
            for b in range(NBLK):
                nc.sync.dma_start(out=z_d[b * 128:(b + 1) * 128, :],
                                  in_=t1_all[:, b, :])

        for p in (psC, pC, psB, pBs, pB):
            p.release()
        cpool.release()

    nc.compile()
    return nc


def kernel(**inputs):
    import os
    from concourse.bass_utils import run_bass_kernel_spmd

    host = _build_host_data(inputs)
    nc = _build_program(host)

    in_maps = []
    for c in range(NCORES):
        m = {
            "xT": host["xT"],
            "x_own": host["x_own"][c],
            "x_ownT": host["x_ownT"][c],
            "Wp": host["Wp"], "Wad": host["Wad"],
            "I128": host["I128"], "iota8": host["iota8"],
            "W1": host["W1"], "W2": host["W2"], "b1col": host["b1col"],
        }
        if not host["triv_bgat"]:
            m["bgat_r"] = np.tile(host["bias_gat"].reshape(1, -1), (128, 1))
        if not host["triv_bff2"]:
            m["bff2_r"] = np.tile(host["b_ff2"].reshape(1, -1), (128, 1))
        if not host["triv_gb1"]:
            m["g1_r"] = np.tile(host["gamma1"].reshape(1, -1), (128, 1))
            m["b1_r"] = np.tile(host["beta1"].reshape(1, -1), (128, 1))
        if not host["triv_gb2"]:
            m["g2_r"] = np.tile(host["gamma2"].reshape(1, -1), (128, 1))
            m["b2_r"] = np.tile(host["beta2"].reshape(1, -1), (128, 1))
        for key in ("gidx_al", "gidx_ah", "gidx_sl", "gidx_sh",
                    "aidx_sl", "aidx_sh", "dl_sl", "dl_sh"):
            if key in host["per_core"][c]:
                m[key] = host["per_core"][c][key]
        in_maps.append(m)

    trace = bool(int(os.environ.get("GAT_TRACE", "0")))
    res = run_bass_kernel_spmd(nc, in_maps, core_ids=list(range(NCORES)),
                               trace=trace)
    if trace and res.exec_time_ns:
        print(f"HW exec time: {res.exec_time_ns} ns")
    if bool(int(os.environ.get("GAT_TIME", "0"))):
        try:
            from concourse.timeline_sim import TimelineSim
            ts = TimelineSim(nc)
            dur = ts.simulate()
            print(f"HW exec time: {dur:.0f} ns (cost-model timeline estimate)")
        except Exception as e:
            print("timeline sim failed:", e)

    out = np.zeros((N, D), np.float32)
    for c in range(NCORES):
        lo_n = OWN * c
        hi_n = min(OWN * (c + 1), N)
        out[lo_n:hi_n] = res.results[c]["z"][: hi_n - lo_n]
    return out


# revision 18
# speedup vs baseline: 1.3448x; 1.0223x over previous
"""Trainium2 Bass kernel for a GAT block (GATConv + LN + FFN + LN).

v2: partition-aligned destination scheme.

Per-core plan (identity node order; core c owns nodes [OWN*c, OWN*(c+1))):
  Phase A: hp = x @ [W | W@Asrc | W@Adst] for all 50176 nodes ([128,144] psum
           per tile); rows [h(128) | a_src(8) | pad] stored to h_d (512B bf16
           rows, +1 row offset; row 0 / row NP+1 are pad rows with
           a_src = -200 so padded gather slots contribute exp(...)=~0).
           a_dst for own nodes kept in SBUF (ad_sb [128, NBLK, 8] bf16) and
           also written to ad_d [OWN,128] for the spill path.
  Phase B: edges partitioned by dst-block (128 own nodes per block).
           ALIGNED streams (AL/AH = src row below/above 32767): granule g of
           block b holds, at partition p, the g-th in-edge of node p. One
           512B-row dma_gather per slot brings [h | a_src]; a_dst is read
           directly from ad_sb (lane == dst). p = exp(leaky(a_src+a_dst));
           pexp (p broadcast over F) is written by the Act engine into the
           pad half of the gathered rows so msg = h*pexp is a fully-packed
           bf16 DVE op (2x mode). Scatter = psum accumulate with lhsT=I.
           SPILL streams (SL/SH): per-(block,stream) overflow edges beyond
           the per-lane cap, dst-grouped; S built via is_equal with a
           middle-broadcast AP (2x mode), a_dst gathered from ad_d;
           scatter matmul with lhsT = S[:, :, g].
  Phase C: deferred, batched: per-block bn_stats/apply with the two Sqrt ops
           batched over all blocks (2 act-table loads total).
"""
import numpy as np
import ml_dtypes

N = 50000
NCORES = 8
OWN = 6272             # nodes per core (49 blocks of 128)
NP = OWN * NCORES      # 50176
BLK = 128
NBLK = OWN // BLK      # 49
H, F, D = 8, 16, 128
LN_EPS = 1e-5

LO_SPLIT = 32767       # src node < LO_SPLIT -> lo stream (h_d row = src+1)
HD_ROWS = 50432        # h_d rows: 0 pad_lo, 1..NP nodes, NP+1 pad_hi
PAD_HI_IDX = NP + 1 - 32768   # 17409
AL_GPC = 24            # granules per aligned chunk (3072 slots)
SP_GPC = 8             # granules per spill chunk (1024 slots)
KAL, KSP = 2.2, 4.7    # per-slot cost weights for cap optimization
PAD_DL = 200.0

bf16 = ml_dtypes.bfloat16


def _wrap16(idx):
    L = idx.shape[0]
    w = idx.reshape(L // 16, 16).T.astype(np.int16)
    return np.tile(w, (8, 1))                      # [128, L/16]


def _bfr(x):
    return np.ascontiguousarray(np.asarray(x, dtype=np.float32).astype(bf16))


def _build_host_data(inputs):
    x = np.asarray(inputs["x"], np.float32)
    W = np.asarray(inputs["W_gat"], np.float32)
    att_src = np.asarray(inputs["att_src"], np.float32)
    att_dst = np.asarray(inputs["att_dst"], np.float32)
    ei = np.asarray(inputs["edge_index"])

    src = ei[0].astype(np.int64)
    dst = ei[1].astype(np.int64)
    loops = np.arange(N, dtype=np.int64)
    src = np.concatenate([src, loops])
    dst = np.concatenate([dst, loops])

    # ---- per-core edge tables (identity node order) ----
    deg = np.zeros((NCORES, NBLK, BLK, 2), dtype=np.int32)
    core_e = []
    for c in range(NCORES):
        m = (dst >= OWN * c) & (dst < min(OWN * (c + 1), N))
        e_src = src[m]
        d_l = dst[m] - OWN * c
        b = d_l >> 7
        lane = d_l & 127
        s = (e_src >= LO_SPLIT).astype(np.int64)
        np.add.at(deg, (c, b, lane, s), 1)
        core_e.append((e_src, d_l, b, lane, s))

    # ---- shared caps per (block, stream); spill granule profile ----
    caps = np.zeros((NBLK, 2), dtype=np.int64)
    spg = np.zeros((NBLK, 2), dtype=np.int64)
    for b in range(NBLK):
        for s in range(2):
            d = deg[:, b, :, s]                       # [cores, 128]
            dmax = int(d.max())
            caps_r = np.arange(dmax + 1)
            spill = np.maximum(d[:, :, None] - caps_r[None, None, :], 0
                               ).sum(axis=1).max(axis=0)     # [dmax+1]
            g_sp = -(-spill // 128)
            cost = 128 * caps_r * KAL + 128 * g_sp * KSP
            k = int(np.argmin(cost))
            caps[b, s] = k
            spg[b, s] = g_sp[k]

    albase = np.zeros((NBLK, 2), dtype=np.int64)
    spbase = np.zeros((NBLK, 2), dtype=np.int64)
    albase[1:] = np.cumsum(caps[:-1], axis=0)
    spbase[1:] = np.cumsum(spg[:-1], axis=0)
    G_AL = [int(caps[:, s].sum()) for s in range(2)]
    G_SP = [int(spg[:, s].sum()) for s in range(2)]
    # pad granule counts to chunk multiples
    G_ALp = [-(-g // AL_GPC) * AL_GPC if g else 0 for g in G_AL]
    G_SPp = [-(-g // SP_GPC) * SP_GPC if g else 0 for g in G_SP]
    L_AL = [g * 128 for g in G_ALp]
    L_SP = [g * 128 for g in G_SPp]

    # block-of-granule maps (shared); pad granules -> block 0
    blk_of_g = []
    for s in range(2):
        bg = np.repeat(np.arange(NBLK), caps[:, s])
        bg = np.concatenate([bg, np.zeros(G_ALp[s] - len(bg), dtype=np.int64)])
        blk_of_g.append(bg)
    # runs per aligned chunk: list of (g0_local, g1_local, block)
    runs = []
    for s in range(2):
        rs = []
        bg = blk_of_g[s]
        for k in range(G_ALp[s] // AL_GPC):
            seg = bg[k * AL_GPC:(k + 1) * AL_GPC]
            r = []
            i = 0
            while i < AL_GPC:
                j = i
                while j < AL_GPC and seg[j] == seg[i]:
                    j += 1
                r.append((i, j, int(seg[i])))
                i = j
            rs.append(r)
        runs.append(rs)

    # ---- per-core slot data ----
    per_core = []
    for c in range(NCORES):
        e_src, d_l, b, lane, s = core_e[c]
        key = (b * 128 + lane) * 2 + s
        order = np.argsort(key, kind="stable")
        ks = key[order]
        chg = np.r_[True, ks[1:] != ks[:-1]] if len(ks) else np.array([], bool)
        grp_id = np.cumsum(chg) - 1 if len(ks) else ks
        starts = np.flatnonzero(chg)
        occ = np.arange(len(ks)) - starts[grp_id] if len(ks) else ks
        so, bo, lo_, eo = s[order], b[order], lane[order], e_src[order]
        dlo = d_l[order]
        cap_e = caps[bo, so]
        al_m = occ < cap_e

        enc = np.where(so == 0, eo + 1, eo + 1 - 32768).astype(np.int64)

        gidx_al = [np.zeros(L_AL[0], dtype=np.int64),
                   np.full(L_AL[1], PAD_HI_IDX, dtype=np.int64)]
        for s_ in range(2):
            mm = al_m & (so == s_)
            g = albase[bo[mm], s_] + occ[mm]
            slot = g * 128 + lo_[mm]
            gidx_al[s_][slot] = enc[mm]

        gidx_sp = [np.zeros(L_SP[0], dtype=np.int64),
                   np.full(L_SP[1], PAD_HI_IDX, dtype=np.int64)]
        dl_sp = [np.full(L_SP[s_], PAD_DL, dtype=np.float32) for s_ in range(2)]
        aidx_sp = [np.zeros(L_SP[s_], dtype=np.int64) for s_ in range(2)]
        sp_m = ~al_m
        key2 = bo[sp_m] * 2 + so[sp_m]
        order2 = np.argsort(key2, kind="stable")
        k2 = key2[order2]
        chg2 = np.r_[True, k2[1:] != k2[:-1]] if len(k2) else np.array([], bool)
        gid2 = np.cumsum(chg2) - 1 if len(k2) else k2
        st2 = np.flatnonzero(chg2)
        rank2 = np.arange(len(k2)) - st2[gid2] if len(k2) else k2
        b2 = bo[sp_m][order2]
        s2 = so[sp_m][order2]
        e2 = enc[sp_m][order2]
        lane2 = lo_[sp_m][order2]
        dl2 = dlo[sp_m][order2]
        for s_ in range(2):
            mm = s2 == s_
            slot = spbase[b2[mm], s_] * 128 + rank2[mm]
            gidx_sp[s_][slot] = e2[mm]
            dl_sp[s_][slot] = lane2[mm].astype(np.float32)
            aidx_sp[s_][slot] = dl2[mm]

        ent = {}
        for s_, nm in ((0, "l"), (1, "h")):
            if L_AL[s_]:
                ent[f"gidx_a{nm}"] = _wrap16(gidx_al[s_])
            if L_SP[s_]:
                ent[f"gidx_s{nm}"] = _wrap16(gidx_sp[s_])
                ent[f"aidx_s{nm}"] = _wrap16(aidx_sp[s_])
                ent[f"dl_s{nm}"] = np.ascontiguousarray(
                    dl_sp[s_].astype(bf16).reshape(-1, 128).T)
        per_core.append(ent)

    # ---- weights / constants ----
    Asrc = np.zeros((D, H), np.float32)
    Adst = np.zeros((D, H), np.float32)
    for h in range(H):
        Asrc[h * F:(h + 1) * F, h] = att_src[h]
        Adst[h * F:(h + 1) * F, h] = att_dst[h]
    Wp = _bfr(np.concatenate([W, W @ Asrc], axis=1))             # [128,136]
    Wad = _bfr(W @ Adst)                                         # [128,8]
    I128 = _bfr(np.eye(128, dtype=np.float32))
    iota8 = _bfr(np.tile(np.arange(BLK, dtype=np.float32)[None, :, None],
                         (128, 1, SP_GPC)).reshape(128, BLK * SP_GPC))

    xp = np.zeros((NP, D), np.float32)
    xp[:N] = x
    xT = np.ascontiguousarray(xp.T.astype(bf16))                 # [128, NP]
    x_own = [np.ascontiguousarray(xp[OWN * c: OWN * (c + 1)]) for c in range(NCORES)]
    x_ownT = [np.ascontiguousarray(xp[OWN * c: OWN * (c + 1)].T.astype(bf16))
              for c in range(NCORES)]

    host = {
        "caps": caps, "spg": spg, "albase": albase, "spbase": spbase,
        "G_ALp": G_ALp, "G_SPp": G_SPp, "L_AL": L_AL, "L_SP": L_SP,
        "runs": runs, "per_core": per_core,
        "xT": xT, "x_own": x_own, "x_ownT": x_ownT, "Wp": Wp, "Wad": Wad,
        "I128": I128, "iota8": iota8,
        "W1": _bfr(np.asarray(inputs["w_ff1"], np.float32)),     # [128,256]
        "W2": _bfr(np.asarray(inputs["w_ff2"], np.float32)),     # [256,128]
        "b1col": np.ascontiguousarray(
            np.asarray(inputs["b_ff1"], np.float32).reshape(2, 128).T),  # [128,2]
    }
    host["bias_gat"] = np.asarray(inputs["bias_gat"], np.float32)
    host["b_ff2"] = np.asarray(inputs["b_ff2"], np.float32)
    for nm in ("gamma1", "beta1", "gamma2", "beta2"):
        host[nm] = np.asarray(inputs[nm], np.float32)
    host["triv_gb1"] = bool(np.all(host["gamma1"] == 1) and np.all(host["beta1"] == 0))
    host["triv_gb2"] = bool(np.all(host["gamma2"] == 1) and np.all(host["beta2"] == 0))
    host["triv_bgat"] = bool(np.all(host["bias_gat"] == 0))
    host["triv_bff2"] = bool(np.all(host["b_ff2"] == 0))
    return host


def _build_program(host):
    import os as _os
    import concourse.bacc as bacc
    import concourse.mybir as mybir
    import concourse.tile as tile
    from concourse.bass import AP

    fp32 = mybir.dt.float32
    bft = mybir.dt.bfloat16
    i16 = mybir.dt.int16
    Alu = mybir.AluOpType
    Act = mybir.ActivationFunctionType

    caps, spg = host["caps"], host["spg"]
    albase, spbase = host["albase"], host["spbase"]
    L_AL, L_SP = host["L_AL"], host["L_SP"]
    runs = host["runs"]

    nc = bacc.Bacc("TRN2")

    # ---- DRAM tensors ----
    xT_d = nc.dram_tensor("xT", [128, NP], bft, kind="ExternalInput")
    xown_d = nc.dram_tensor("x_own", [OWN, D], fp32, kind="ExternalInput")
    Wp_d = nc.dram_tensor("Wp", [128, 136], bft, kind="ExternalInput")
    Wad_d = nc.dram_tensor("Wad", [128, 8], bft, kind="ExternalInput")
    xoT_d = nc.dram_tensor("x_ownT", [128, OWN], bft, kind="ExternalInput")
    I128_d = nc.dram_tensor("I128", [128, 128], bft, kind="ExternalInput")
    iota8_d = nc.dram_tensor("iota8", [128, BLK * SP_GPC], bft, kind="ExternalInput")
    W1_d = nc.dram_tensor("W1", [128, 256], bft, kind="ExternalInput")
    W2_d = nc.dram_tensor("W2", [256, 128], bft, kind="ExternalInput")
    b1c_d = nc.dram_tensor("b1col", [128, 2], fp32, kind="ExternalInput")
    gl_d = {}
    if not host["triv_bgat"]:
        gl_d["bgat"] = nc.dram_tensor("bgat_r", [128, 128], fp32, kind="ExternalInput")
    if not host["triv_bff2"]:
        gl_d["bff2"] = nc.dram_tensor("bff2_r", [128, 128], fp32, kind="ExternalInput")
    if not host["triv_gb1"]:
        gl_d["g1"] = nc.dram_tensor("g1_r", [128, 128], fp32, kind="ExternalInput")
        gl_d["b1"] = nc.dram_tensor("b1_r", [128, 128], fp32, kind="ExternalInput")
    if not host["triv_gb2"]:
        gl_d["g2"] = nc.dram_tensor("g2_r", [128, 128], fp32, kind="ExternalInput")
        gl_d["b2"] = nc.dram_tensor("b2_r", [128, 128], fp32, kind="ExternalInput")

    sd = {}
    for s, nm in ((0, "l"), (1, "h")):
        if L_AL[s]:
            sd[f"gidx_a{nm}"] = nc.dram_tensor(
                f"gidx_a{nm}", [128, L_AL[s] // 16], i16, kind="ExternalInput")
        if L_SP[s]:
            sd[f"gidx_s{nm}"] = nc.dram_tensor(
                f"gidx_s{nm}", [128, L_SP[s] // 16], i16, kind="ExternalInput")
            sd[f"aidx_s{nm}"] = nc.dram_tensor(
                f"aidx_s{nm}", [128, L_SP[s] // 16], i16, kind="ExternalInput")
            sd[f"dl_s{nm}"] = nc.dram_tensor(
                f"dl_s{nm}", [128, L_SP[s] // 128], bft, kind="ExternalInput")

    h_d = nc.dram_tensor("h_scratch", [HD_ROWS, 256], bft, kind="Internal")
    ad_d = nc.dram_tensor("adst_scratch", [OWN, 128], bft, kind="Internal")
    z_d = nc.dram_tensor("z", [OWN, D], fp32, kind="ExternalOutput")

    h_lo = h_d[0:32768, :]
    h_hi = h_d[32768:HD_ROWS, :]

    NT = NP // 128                    # 392 node tiles
    with tile.TileContext(nc) as tc:
        # ================= consts =================
        cpool = tc.alloc_tile_pool(name="consts", bufs=1)
        Wp_s = cpool.tile([128, 136], bft)
        nc.sync.dma_start(out=Wp_s[:], in_=Wp_d[:])
        Wad_s = cpool.tile([128, 8], bft)
        nc.sync.dma_start(out=Wad_s[:], in_=Wad_d[:])
        I128_s = cpool.tile([128, 128], bft)
        nc.sync.dma_start(out=I128_s[:], in_=I128_d[:])
        iota8_s = cpool.tile([128, BLK, SP_GPC], bft)
        nc.sync.dma_start(out=iota8_s[:], in_=iota8_d[:].rearrange(
            "p (n g) -> p n g", g=SP_GPC))
        W1_s = cpool.tile([128, 256], bft)
        nc.sync.dma_start(out=W1_s[:], in_=W1_d[:])
        W2_s = cpool.tile([256 // 2, 2, 128], bft)
        nc.sync.dma_start(out=W2_s[:],
                          in_=W2_d[:].rearrange("(k h) f -> h k f", k=2))
        b1c_s = cpool.tile([128, 2], fp32)
        nc.sync.dma_start(out=b1c_s[:], in_=b1c_d[:])
        gl_s = {}
        for k, dref in gl_d.items():
            gl_s[k] = cpool.tile([128, 128], fp32, tag=f"gl_{k}")
            nc.sync.dma_start(out=gl_s[k][:], in_=dref[:])
        eps_s = cpool.tile([128, 1], fp32)
        nc.vector.memset(eps_s[:], LN_EPS)
        ad_sb = cpool.tile([128, NBLK, 8], bft)

        # pad rows for h_d (row 0 and row NP+1): zeros, a_src cols = -200
        padr = cpool.tile([128, 256], bft)
        nc.vector.memset(padr[:], 0.0)
        nc.vector.memset(padr[:, 128:136], -200.0)
        nc.sync.dma_start(out=h_d[0:1, :], in_=padr[0:1, :])
        nc.sync.dma_start(out=h_d[NP + 1:NP + 2, :], in_=padr[0:1, :])

        # persistent phase-C tiles
        xo_all = cpool.tile([128, NBLK, 128], fp32)   # xo -> t1 -> t2 (reused)
        u_all = cpool.tile([128, NBLK, 128], fp32)
        mv1 = cpool.tile([128, NBLK, 2], fp32)
        mv2 = cpool.tile([128, NBLK, 2], fp32)
        sc1 = cpool.tile([128, NBLK], fp32)
        sc2 = cpool.tile([128, NBLK], fp32)
        nc.sync.dma_start(out=xo_all[:],
                          in_=xown_d[:].rearrange("(j n) d -> n j d", j=NBLK))

        # ================= phase A =================
        with tc.tile_pool(name="pA", bufs=4) as pA, \
             tc.tile_pool(name="psA", bufs=4, space="PSUM") as psA:
            # per-core a_dst of own nodes first: x_ownT @ Wad
            GT2 = 7
            xoT = pA.tile([128, OWN], bft, tag="xoT")
            nc.sync.dma_start(out=xoT[:], in_=xoT_d[:])
            for t0 in range(0, NBLK, GT2):
                ps2 = psA.tile([128, GT2, 8], fp32, tag="psA2")
                for j in range(GT2):
                    t = t0 + j
                    nc.tensor.matmul(ps2[:, j, :],
                                     lhsT=xoT[:, t * 128:(t + 1) * 128],
                                     rhs=Wad_s[:], start=True, stop=True)
                nc.vector.tensor_copy(out=ad_sb[:, t0:t0 + GT2, :], in_=ps2[:])
                nc.sync.dma_start(
                    out=ad_d[t0 * 128:(t0 + GT2) * 128, 0:8].rearrange(
                        "(j n) d -> n j d", j=GT2),
                    in_=ad_sb[:, t0:t0 + GT2, :])

            GT = 3
            SGT = 6
            XB = 12
            xt = None
            stage = None
            for tg in range((NT + GT - 1) // GT):
                t0 = tg * GT
                ntl = min(GT, NT - t0)
                if t0 % XB == 0:
                    nxb = min(XB, NT - t0)
                    xt = pA.tile([128, XB * 128], bft, tag="xt")
                    nc.scalar.dma_start(out=xt[:, :nxb * 128],
                                        in_=xT_d[:, t0 * 128:(t0 + nxb) * 128])
                ps = psA.tile([128, GT, 136], fp32, tag="psA")
                for j in range(ntl):
                    jo = (t0 % XB) + j
                    nc.tensor.matmul(ps[:, j, :],
                                     lhsT=xt[:, jo * 128:(jo + 1) * 128],
                                     rhs=Wp_s[:], start=True, stop=True)
                so = t0 % SGT
                if so == 0:
                    stage = pA.tile([128, SGT, 256], bft, tag="stage")
                if tg % 2 == 0:
                    nc.scalar.activation(out=stage[:, so:so + ntl, 0:136],
                                         in_=ps[:, :ntl, :], func=Act.Copy)
                else:
                    nc.vector.tensor_copy(out=stage[:, so:so + ntl, 0:136],
                                          in_=ps[:, :ntl, :])
                if so + ntl == SGT or t0 + ntl == NT:
                    s0 = t0 - so
                    nst = so + ntl
                    nc.sync.dma_start(
                        out=h_d[s0 * 128 + 1:(s0 + nst) * 128 + 1, :].rearrange(
                            "(j n) d -> n j d", j=nst),
                        in_=stage[:, :nst, :])

        tc.strict_bb_all_engine_barrier()

        dbg = _os.environ.get("GAT_DBG")
        if dbg in ("h", "ae"):
            with tc.tile_pool(name="dbg", bufs=2) as dp:
                for t in range(NBLK):
                    dt_ = dp.tile([128, 256], bft, tag="d")
                    nc.sync.dma_start(out=dt_[:],
                                      in_=h_d[1 + t * 128: 1 + (t + 1) * 128, :])
                    df = dp.tile([128, 128], fp32, tag="df")
                    if dbg == "h":
                        nc.vector.tensor_copy(out=df[:], in_=dt_[:, 0:128])
                    else:
                        nc.vector.memset(df[:], 0.0)
                        nc.vector.tensor_copy(out=df[:, 0:16], in_=dt_[:, 128:144])
                    nc.sync.dma_start(out=z_d[t * 128:(t + 1) * 128, :], in_=df[:])
            _finish_dbg = True
        else:
            _finish_dbg = False

        # ================= phase B =================
        pB = tc.alloc_tile_pool(name="pB", bufs=2)
        pBs = tc.alloc_tile_pool(name="pBsmall", bufs=4)
        psB = tc.alloc_tile_pool(name="psB", bufs=2, space="PSUM")
        pC = tc.alloc_tile_pool(name="pC", bufs=2)
        psC = tc.alloc_tile_pool(name="psC", bufs=2, space="PSUM")

        SCFG = {
            "AL": dict(gpc=AL_GPC, al=True, src=h_lo, gx="gidx_al", s=0),
            "AH": dict(gpc=AL_GPC, al=True, src=h_hi, gx="gidx_ah", s=1),
            "SL": dict(gpc=SP_GPC, al=False, src=h_lo, gx="gidx_sl",
                       ax="aidx_sl", dl="dl_sl", s=0),
            "SH": dict(gpc=SP_GPC, al=False, src=h_hi, gx="gidx_sh",
                       ax="aidx_sh", dl="dl_sh", s=1),
        }
        chunk_tiles = {st: {} for st in SCFG}

        def emit_chunk(st, k):
            ct = chunk_tiles[st]
            if k in ct:
                return ct[k]
            cfg = SCFG[st]
            gpc = cfg["gpc"]
            ch = gpc * 128
            nch = sd[cfg["gx"]].shape[1] // (ch // 16)   # total chunks
            k4 = k // 4
            gx4 = cfg.setdefault("_gx4", {})
            if k4 not in gx4:
                c0 = k4 * 4 * (ch // 16)
                c1 = min((k4 + 1) * 4 * (ch // 16), sd[cfg["gx"]].shape[1])
                t4 = pBs.tile([128, 4 * (ch // 16)], i16, tag=f"gix{st}")
                nc.scalar.dma_start(out=t4[:, :c1 - c0],
                                    in_=sd[cfg["gx"]][:, c0:c1])
                gx4[k4] = t4
            gix = gx4[k4][:, (k % 4) * (ch // 16):(k % 4 + 1) * (ch // 16)]
            h_ch = pB.tile([128, gpc, 256], bft, tag=f"h{st}")
            nc.gpsimd.dma_gather(h_ch[:], cfg["src"], gix, ch, ch, 256,
                                 single_packet=False)
            eL = pBs.tile([128, gpc, 8], bft, tag=f"eL{st}")
            res = {}
            if cfg["al"]:
                for (g0, g1, b) in runs[cfg["s"]][k]:
                    sl = ad_sb[:, b, :]
                    ad_b = AP(sl.tensor, sl.offset,
                              [sl.ap[0], [0, g1 - g0], sl.ap[1]])
                    nc.vector.tensor_tensor(out=eL[:, g0:g1, :],
                                            in0=h_ch[:, g0:g1, 128:136],
                                            in1=ad_b, op=Alu.add)
            else:
                ax4 = cfg.setdefault("_ax4", {})
                if k4 not in ax4:
                    c0 = k4 * 4 * (ch // 16)
                    c1 = min((k4 + 1) * 4 * (ch // 16), sd[cfg["ax"]].shape[1])
                    t4 = pBs.tile([128, 4 * (ch // 16)], i16, tag=f"aix{st}")
                    nc.scalar.dma_start(out=t4[:, :c1 - c0],
                                        in_=sd[cfg["ax"]][:, c0:c1])
                    ax4[k4] = t4
                aix = ax4[k4][:, (k % 4) * (ch // 16):(k % 4 + 1) * (ch // 16)]
                adE = pB.tile([128, gpc, 128], bft, tag=f"adE{st}")
                nc.gpsimd.dma_gather(adE[:], ad_d[:], aix, ch, ch, 128,
                                     single_packet=False)
                nc.vector.tensor_tensor(out=eL[:], in0=h_ch[:, :, 128:136],
                                        in1=adE[:, :, 0:8], op=Alu.add)
                dl4 = cfg.setdefault("_dl4", {})
                if k4 not in dl4:
                    c0 = k4 * 4 * gpc
                    c1 = min((k4 + 1) * 4 * gpc, sd[cfg["dl"]].shape[1])
                    t4 = pBs.tile([128, 4 * gpc], bft, tag=f"dl{st}")
                    nc.scalar.dma_start(out=t4[:, :c1 - c0],
                                        in_=sd[cfg["dl"]][:, c0:c1])
                    dl4[k4] = t4
                dlt = dl4[k4][:, (k % 4) * gpc:(k % 4 + 1) * gpc]
                S2 = pB.tile([128, BLK, gpc], bft, tag=f"S2{st}")
                dsl = dlt
                dl_b = AP(dsl.tensor, dsl.offset,
                          [dsl.ap[0], [0, BLK], dsl.ap[1]])
                nc.vector.tensor_tensor(out=S2[:], in0=dl_b,
                                        in1=iota8_s[:, :, 0:gpc],
                                        op=Alu.is_equal)
                res["S2"] = S2
            eL2 = pBs.tile([128, gpc, 8], bft, tag=f"eL2{st}")
            nc.vector.scalar_tensor_tensor(out=eL2[:], in0=eL[:], scalar=0.2,
                                           in1=eL[:], op0=Alu.mult, op1=Alu.max)
            msgp = pB.tile([128, gpc, 136], bft, tag=f"msgp{st}")
            nc.scalar.activation(out=msgp[:, :, 128:136], in_=eL2[:],
                                 func=Act.Exp)
            if bool(int(_os.environ.get("GAT_NO_PEXP", "0"))):
                nc.vector.tensor_tensor(
                    out=msgp[:, :, 0:128].rearrange("p g (h f) -> p g h f", f=F),
                    in0=h_ch[:, :, 0:128].rearrange("p g (h f) -> p g h f", f=F),
                    in1=msgp[:, :, 128:136].to_broadcast([128, gpc, 8, F]),
                    op=Alu.mult)
            else:
                nc.scalar.activation(
                    out=h_ch[:, :, 128:256].rearrange("p g (h f) -> p g h f", f=F),
                    in_=eL2[:].to_broadcast([128, gpc, 8, F]), func=Act.Exp)
                nc.vector.tensor_tensor(out=msgp[:, :, 0:128],
                                        in0=h_ch[:, :, 0:128],
                                        in1=h_ch[:, :, 128:256], op=Alu.mult)
            res["msgp"] = msgp
            ct[k] = res
            return res

        GRPS = [0, 12, 24, 34, 42, 47, NBLK]
        GRP_END = {GRPS[i + 1]: GRPS[i] for i in range(len(GRPS) - 1)}
        t1_all = xo_all

        def emit_group_c(b0, b1):
            nb = b1 - b0
            nc.scalar.activation(out=sc1[:, b0:b1], in_=mv1[:, b0:b1, 1],
                                 func=Act.Sqrt, bias=eps_s[:])
            nc.vector.reciprocal(out=sc1[:, b0:b1], in_=sc1[:, b0:b1])
            for b in range(b0, b1):
                u = u_all[:, b, :]
                nc.vector.tensor_scalar(out=u, in0=t1_all[:, b, :],
                                        scalar1=mv1[:, b, 0:1], op0=Alu.subtract,
                                        scalar2=sc1[:, b:b + 1], op1=Alu.mult)
                if not host["triv_gb1"]:
                    nc.vector.tensor_tensor(out=u, in0=u, in1=gl_s["g1"][:],
                                            op=Alu.mult)
                    nc.vector.tensor_tensor(out=u, in0=u, in1=gl_s["b1"][:],
                                            op=Alu.add)
                u_bf = pC.tile([128, 128], bft, tag="ubf")
                nc.gpsimd.tensor_copy(out=u_bf[:], in_=u)
                uT_ps = psC.tile([128, 128], bft, tag="uT")
                nc.tensor.transpose(uT_ps[:], in_=u_bf[:], identity=I128_s[:])
                uT = pC.tile([128, 128], bft, tag="uTs")
                nc.scalar.activation(out=uT[:], in_=uT_ps[:], func=Act.Copy)
                f1ps = psC.tile([128, 2, 128], fp32, tag="f1")
                for j in range(2):
                    nc.tensor.matmul(f1ps[:, j, :],
                                     lhsT=W1_s[:, j * 128:(j + 1) * 128],
                                     rhs=uT[:], start=True, stop=True)
                r1 = pC.tile([128, 2, 128], bft, tag="r1")
                nc.vector.tensor_scalar(out=r1[:, 0, :], in0=f1ps[:, 0, :],
                                        scalar1=b1c_s[:, 0:1], op0=Alu.add,
                                        scalar2=0.0, op1=Alu.max)
                nc.scalar.activation(out=r1[:, 1, :], in_=f1ps[:, 1, :],
                                     func=Act.Relu, bias=b1c_s[:, 1:2])
                zps = psC.tile([128, 128], fp32, tag="zp")
                for j in range(2):
                    nc.tensor.matmul(zps[:], lhsT=r1[:, j, :], rhs=W2_s[:, j, :],
                                     start=(j == 0), stop=(j == 1))
                t2 = t1_all[:, b, :]
                nc.vector.tensor_tensor(out=t2, in0=u, in1=zps[:], op=Alu.add)
                if not host["triv_bff2"]:
                    nc.vector.tensor_tensor(out=t2, in0=t2, in1=gl_s["bff2"][:],
                                            op=Alu.add)
                bst = pBs.tile([128, 6], fp32, tag="bst")
                nc.vector.bn_stats(out=bst[:], in_=t2)
                nc.vector.bn_aggr(out=mv2[:, b, :], in_=bst[:])
            nc.scalar.activation(out=sc2[:, b0:b1], in_=mv2[:, b0:b1, 1],
                                 func=Act.Sqrt, bias=eps_s[:])
            nc.vector.reciprocal(out=sc2[:, b0:b1], in_=sc2[:, b0:b1])
            for b in range(b0, b1):
                zt = u_all[:, b, :]
                nc.vector.tensor_scalar(out=zt, in0=t1_all[:, b, :],
                                        scalar1=mv2[:, b, 0:1], op0=Alu.subtract,
                                        scalar2=sc2[:, b:b + 1], op1=Alu.mult)
                if not host["triv_gb2"]:
                    nc.vector.tensor_tensor(out=zt, in0=zt, in1=gl_s["g2"][:],
                                            op=Alu.mult)
                    nc.vector.tensor_tensor(out=zt, in0=zt, in1=gl_s["b2"][:],
                                            op=Alu.add)
            nc.scalar.dma_start(
                out=z_d[b0 * 128:b1 * 128, :].rearrange("(j n) d -> n j d", j=nb),
                in_=u_all[:, b0:b1, :])

        no_spill = bool(int(_os.environ.get("GAT_NO_SPILL", "0")))
        no_al = bool(int(_os.environ.get("GAT_NO_AL", "0")))
        no_c = bool(int(_os.environ.get("GAT_NO_C", "0")))
        if _finish_dbg or dbg in ("den", "gt"):
            no_c = True
        for b in range(NBLK if not _finish_dbg else 0):
            glist = []
            if not no_al:
                for s, st in ((0, "AL"), (1, "AH")):
                    for o in range(int(caps[b, s])):
                        glist.append((st, int(albase[b, s]) + o))
            if not no_spill:
                for s, st in ((0, "SL"), (1, "SH")):
                    for o in range(int(spg[b, s])):
                        glist.append((st, int(spbase[b, s]) + o))
            tot = len(glist)
            ps_blk = psB.tile([128, 136], fp32, tag="blk")
            for i, (st, gg) in enumerate(glist):
                cfg = SCFG[st]
                res = emit_chunk(st, gg // cfg["gpc"])
                gl = gg % cfg["gpc"]
                lhsT = I128_s[:] if cfg["al"] else res["S2"][:, :, gl]
                nc.tensor.matmul(ps_blk[:], lhsT=lhsT,
                                 rhs=res["msgp"][:, gl, :],
                                 start=(i == 0), stop=(i == tot - 1))
            # normalize: gt = agg/denom; t1 = x + gt (into xo_all)
            if dbg == "den":
                den = pBs.tile([128, 128], fp32, tag="den")
                nc.vector.memset(den[:], 0.0)
                nc.vector.tensor_copy(out=den[:, 0:8], in_=ps_blk[:, 128:136])
                nc.sync.dma_start(out=z_d[b * 128:(b + 1) * 128, :], in_=den[:])
                continue
            rec = pBs.tile([128, 8], fp32, tag="rec")
            nc.vector.reciprocal(out=rec[:], in_=ps_blk[:, 128:136])
            gt = pBs.tile([128, 128], fp32, tag="gt")
            nc.vector.tensor_tensor(
                out=gt[:].rearrange("p (h f) -> p h f", f=F),
                in0=ps_blk[:, 0:128].rearrange("p (h f) -> p h f", f=F),
                in1=rec[:].to_broadcast([128, 8, F]), op=Alu.mult)
            if not host["triv_bgat"]:
                nc.vector.tensor_tensor(out=gt[:], in0=gt[:],
                                        in1=gl_s["bgat"][:], op=Alu.add)
            if dbg == "gt":
                nc.sync.dma_start(out=z_d[b * 128:(b + 1) * 128, :], in_=gt[:])
                continue
            nc.vector.tensor_tensor(out=xo_all[:, b, :], in0=xo_all[:, b, :],
                                    in1=gt[:], op=Alu.add)
            if not no_c:
                bst = pBs.tile([128, 6], fp32, tag="bst")
                nc.vector.bn_stats(out=bst[:], in_=xo_all[:, b, :])
                nc.vector.bn_aggr(out=mv1[:, b, :], in_=bst[:])
                if b + 1 in GRP_END:
                    emit_group_c(GRP_END[b + 1], b + 1)

        # ================= phase C emitted group-wise inside the block loop ====
        if no_c and dbg not in ("den", "gt"):
            for b in range(NBLK):
                nc.sync.dma_start(out=z_d[b * 128:(b + 1) * 128, :],
                                  in_=t1_all[:, b, :])

        for p in (psC, pC, psB, pBs, pB):
            p.release()
        cpool.release()

    nc.compile()
    return nc


def kernel(**inputs):
    import os
    from concourse.bass_utils import run_bass_kernel_spmd

    host = _build_host_data(inputs)
    nc = _build_program(host)

    in_maps = []
    for c in range(NCORES):
        m = {
            "xT": host["xT"],
            "x_own": host["x_own"][c],
            "x_ownT": host["x_ownT"][c],
            "Wp": host["Wp"], "Wad": host["Wad"],
            "I128": host["I128"], "iota8": host["iota8"],
            "W1": host["W1"], "W2": host["W2"], "b1col": host["b1col"],
        }
        if not host["triv_bgat"]:
            m["bgat_r"] = np.tile(host["bias_gat"].reshape(1, -1), (128, 1))
        if not host["triv_bff2"]:
            m["bff2_r"] = np.tile(host["b_ff2"].reshape(1, -1), (128, 1))
        if not host["triv_gb1"]:
            m["g1_r"] = np.tile(host["gamma1"].reshape(1, -1), (128, 1))
            m["b1_r"] = np.tile(host["beta1"].reshape(1, -1), (128, 1))
        if not host["triv_gb2"]:
            m["g2_r"] = np.tile(host["gamma2"].reshape(1, -1), (128, 1))
            m["b2_r"] = np.tile(host["beta2"].reshape(1, -1), (128, 1))
        for key in ("gidx_al", "gidx_ah", "gidx_sl", "gidx_sh",
                    "aidx_sl", "aidx_sh", "dl_sl", "dl_sh"):
            if key in host["per_core"][c]:
                m[key] = host["per_core"][c][key]
        in_maps.append(m)

    trace = bool(int(os.environ.get("GAT_TRACE", "0")))
    res = run_bass_kernel_spmd(nc, in_maps, core_ids=list(range(NCORES)),
                               trace=trace)
    if trace and res.exec_time_ns:
        print(f"HW exec time: {res.exec_time_ns} ns")
    if bool(int(os.environ.get("GAT_TIME", "0"))):
        try:
            from concourse.timeline_sim import TimelineSim
            ts = TimelineSim(nc)
            dur = ts.simulate()
            print(f"HW exec time: {dur:.0f} ns (cost-model timeline estimate)")
        except Exception as e:
            print("timeline sim failed:", e)

    out = np.zeros((N, D), np.float32)
    for c in range(NCORES):
        lo_n = OWN * c
        hi_n = min(OWN * (c + 1), N)
        out[lo_n:hi_n] = res.results[c]["z"][: hi_n - lo_n]
    return out
